# revision 1
# baseline (speedup 1.0000x reference)
"""Trainium2 Bass kernel for nn_Decoder_72911364816952.

Strategy (8 NeuronCores, memory-bound regime):
  Phase A (all cores): the dominant HBM traffic is reading the dynamic-weight
    generator matrices Ww (~540 MB fp32). Each core reads a distinct 1/8
    column-shard (host-sliced) and computes the generated per-sample conv
    weights w[b] = lat_new[b] @ Ww_shard for all 4 samples (tiny output).
  AllToAll (per layer): routes every sample's full generated-weight set to the
    core that will run that sample, keeping the program fully SPMD (the sample
    a core processes is determined by the data the collective hands it, not by
    any rank-dependent addressing).
  Phase B (each core, one sample): runs the spatial decoder network entirely
    on-chip: surf/sin/cos wave path, Sobel folded into the g2c matmuls,
    instance norm via bn_stats/bn_aggr, DynaResidualBlock matmuls in bf16 (fp32 PSUM),
    bilinear-2x-upsample + gauss blur as fused shifted vector ops.
  Output: each core writes its sample's (3,64,64) image; host stacks cores 0-3.
"""
import math
import os
import sys

sys.path.insert(0, "/opt/trn_rl_repo")

import numpy as np

# ---------------------------------------------------------------- constants
L = 3
LAT = 256
F = 64
CPE = 32
B = 4
NCORE = 8
SIZES = [16, 32, 64]
HWS = [s * s for s in SIZES]
K_FREQ = 8

# conv name -> (fin, fout); order defines the column layout of the wws blob
CONVS = [
    ("bt", 1024, 64),
    ("c2w", 64, 32),
    ("w2cA", 32, 128),
    ("w2cB", 128, 128),
    ("g2c", 128, 128),
    ("skip", 256, 64),
    ("c1", 256, 128),
    ("c2", 128, 128),
    ("c3", 128, 64),
]
BIASES = [("bt", 64), ("c2w", 32), ("skip", 64), ("c1", 128), ("c2", 128), ("c3", 64)]

OFF = {}
_off = 0
for _n, _fi, _fo in CONVS:
    OFF[_n] = _off
    _off += (_fi // NCORE) * _fo
SCOLS = _off  # 22272
BOFF = {}
for _n, _fo in BIASES:
    BOFF[_n] = _off
    _off += _fo
SCOLS_T = _off  # 22752

E1 = math.e
_ga_raw = math.exp(-0.5)
GA = _ga_raw / (1.0 + 2.0 * _ga_raw)        # normalized 1D gauss edge tap
GRBA = 1.0 / _ga_raw                         # center/edge tap ratio b/a = e^0.5
UPGAUSS_C = 0.5625 * GA * GA                 # 0.75^2 (upsample) * a^2 (gauss)
PI_2 = math.pi / 2.0


def pos_enc_np(size):
    p = np.arange(size, dtype=np.float32)
    feats = []
    for k in range(K_FREQ):
        ang = p * (2.0 ** k) * (2.0 * np.pi / size)
        s, c = np.sin(ang).astype(np.float32), np.cos(ang).astype(np.float32)
        feats += [np.broadcast_to(s[:, None], (size, size)),
                  np.broadcast_to(c[:, None], (size, size)),
                  np.broadcast_to(s[None, :], (size, size)),
                  np.broadcast_to(c[None, :], (size, size))]
    return np.ascontiguousarray(np.stack(feats).reshape(4 * K_FREQ, size * size))


INPUT_SHAPES = [
    ("wws", (L, LAT, SCOLS_T), "bf16"),
    ("latT", (LAT, B), "f32"),
    ("fracW", (L, LAT, LAT), "f32"),
    ("fracb", (L, LAT), "f32"),
    ("leak", (1, 1), "f32"),
    ("seed", (F, 256), "f32"),
    ("wb0", (1024, HWS[0]), "bf16"),
    ("wb1", (1024, HWS[1]), "bf16"),
    ("wb2", (1024, HWS[2]), "bf16"),
    ("cw0", (CPE, HWS[0]), "bf16"),
    ("cw1", (CPE, HWS[1]), "bf16"),
    ("cw2", (CPE, HWS[2]), "bf16"),
    ("imgWT", (F, 3), "f32"),
    ("imgb", (3, 1), "f32"),
]


# ---------------------------------------------------------------- device code
def build_kernel(tc, ins, out_img):
    import concourse.bass as bass
    from concourse import mybir

    nc = tc.nc
    f32 = mybir.dt.float32
    bf16 = mybir.dt.bfloat16
    ALU = mybir.AluOpType
    ACTF = mybir.ActivationFunctionType
    AX = mybir.AxisListType
    ctxs = []

    def pool(name, bufs, space="SBUF"):
        p = tc.tile_pool(name=name, bufs=bufs, space=space)
        ctxs.append(p)
        return p.__enter__()

    def mk(pool_, shape, tag, dt=None, bufs=None):
        return pool_.tile(shape, dt or f32, name=tag, tag=tag, bufs=bufs)

    # pools
    dram = pool("dram", 1, "DRAM")
    psp = pool("psp", 1, "PSUM")          # per-tag bufs: 8 banks total
    big = pool("big", 1)                  # one slot per tag, large (.,hw) tiles
    gw = pool("gw", 1)                    # generated-weight tiles (per-layer tags)
    ab = pool("ab", 3)                    # a1/a2 per-chunk tiles
    sm = pool("sm", 1)                    # small per-partition scalars
    smc = pool("smc", 2)                  # small per-call scalars
    pha = pool("pha", 3)                  # phase-A rhs stream
    stg = pool("stg", 2)                  # phase-A psum->dram staging
    wbp = pool("wbp", 3)                  # wave-bias stream

    def mmr(ps, lhsT, rhs, start, stop):
        # bf16 operands, fp32 PSUM accumulate
        nc.tensor.matmul(ps, lhsT, rhs, start=start, stop=stop)

    def mm32(ps, lhsT, rhs, start, stop):
        nc.tensor.matmul(ps, lhsT, rhs, start=start, stop=stop)

    # ---------------- setup: latT, leak, lat_newT per layer
    latT = []
    for kc in range(2):
        t = mk(sm, [128, B], f"latT{kc}")
        nc.sync.dma_start(t[:], ins["latT"][kc * 128:(kc + 1) * 128, :])
        latT.append(t)

    leak_sb = mk(sm, [1, 1], "leak")
    nc.sync.dma_start(leak_sb[:], ins["leak"][:, :])
    ones64 = mk(sm, [1, F], "ones64")
    nc.vector.memset(ones64[:], 1.0)
    ps_l = mk(psp, [F, 1], "c2")
    mm32(ps_l[:], ones64[:], leak_sb[:], True, True)
    leak64 = mk(sm, [F, 1], "leak64")
    nc.scalar.copy(leak64[:], ps_l[:])

    lat_newT = []  # [l][kc] -> (128, B) tiles
    for l in range(L):
        fb = []
        for kc in range(2):
            t = mk(sm, [128, 1], f"fracb{l}{kc}")
            nc.sync.dma_start(
                t[:], ins["fracb"][l, kc * 128:(kc + 1) * 128].rearrange(
                    "(p u) -> p u", u=1))
            fb.append(t)
        lnT = []
        for mc in range(2):
            ps = mk(psp, [128, B], "c2")
            for kc in range(2):
                fw = mk(sm, [128, 128], "fwtile")
                nc.sync.dma_start(
                    fw[:], ins["fracW"][l, kc * 128:(kc + 1) * 128,
                                        mc * 128:(mc + 1) * 128])
                mm32(ps[:], fw[:], latT[kc][:], kc == 0, kc == 1)
            t = mk(sm, [128, B], f"latnew{l}{mc}", bf16)
            nc.scalar.activation(t[:], ps[:], ACTF.Identity, bias=fb[mc][:])
            lnT.append(t)
        lat_newT.append(lnT)

    # ---------------- phase A + AllToAll (emitted per layer from main loop)
    a2a_out = []

    def phase_a_stream(l):
        contrib = mk(dram, [NCORE, SCOLS_T], f"contrib{l}", bf16)
        gathered = mk(dram, [NCORE, SCOLS_T], f"a2a{l}", bf16)
        c0 = 0
        di = 0
        while c0 < SCOLS_T:
            dc = min(2048, SCOLS_T - c0)
            rhs = [None, None]
            for kc in range(2):
                rhs[kc] = mk(pha, [128, 2048], f"pharhs{kc}", bf16)
                nc.sync.dma_start(rhs[kc][:, :dc],
                                  ins["wws"][l, kc * 128:(kc + 1) * 128,
                                             c0:c0 + dc])
            stage = mk(stg, [B, 2048], "stage", bf16)
            n0 = 0
            while n0 < dc:
                nn = min(512, dc - n0)
                ps = mk(psp, [B, 512], "pa", bufs=1)
                for kc in range(2):
                    mmr(ps[:, :nn], lat_newT[l][kc][:], rhs[kc][:, n0:n0 + nn],
                        kc == 0, kc == 1)
                if di % 2 == 0:
                    nc.scalar.copy(stage[:, n0:n0 + nn], ps[:, :nn])
                else:
                    nc.vector.tensor_copy(stage[:, n0:n0 + nn], ps[:, :nn])
                di += 1
                n0 += nn
            nc.sync.dma_start(contrib[0:4, c0:c0 + dc], stage[:, :dc])
            nc.sync.dma_start(contrib[4:8, c0:c0 + dc], stage[:, :dc])
            c0 += dc
        return contrib, gathered

    def phase_a_cc(contrib, gathered):
        nc.gpsimd.collective_compute(
            "AllToAll", mybir.AluOpType.bypass,
            replica_groups=[list(range(NCORE))],
            ins=[contrib.opt()],
            outs=[gathered.opt()],
        )
        a2a_out.append(gathered)

    # ---------------- phase B (every core: its routed sample)
    def load_w(l, name, fin, fout, split64=False, keep_f32=False):
        """Generated-weight k-chunk tiles from a2a_out[l], converted to bf16.

        split64=True loads a fin=128 conv as two (64, fout) tiles so
        downstream vector ops / separate rhs tiles stay partition-0
        aligned (walrus requires same start partition on all operands).
        """
        g = a2a_out[l]
        rpr = fin // NCORE  # rows per rank
        off = OFF[name]
        tiles = []
        rows = 64 if split64 else min(fin, 128)
        nchunk = fin // rows
        rpc = rows // rpr  # ranks per chunk
        for kc in range(nchunk):
            t = mk(gw, [rows, fout], f"w_{name}{kc}", bf16)
            for u in range(rpc):
                r = kc * rpc + u
                nc.gpsimd.dma_start(
                    t[u * rpr:(u + 1) * rpr, :],
                    g[r, off:off + rpr * fout].rearrange(
                        "(il o) -> il o", o=fout))
            if keep_f32:
                tf = mk(gw, [rows, fout], f"w_{name}{kc}f")
                nc.vector.tensor_copy(tf[:], t[:])
                tiles.append(tf)
            else:
                tiles.append(t)
        return tiles

    def load_b(l, name, fout, tag=None):
        tb = mk(gw, [fout, 1], (tag or f"b_{name}") + "_h", bf16)
        nc.gpsimd.dma_start(
            tb[:], a2a_out[l][0, BOFF[name]:BOFF[name] + fout].rearrange(
                "(o u) -> o u", u=1))
        t = mk(gw, [fout, 1], tag or f"b_{name}")
        nc.vector.tensor_copy(t[:], tb[:])
        return t

    imgWT = mk(sm, [F, 3], "imgWT")
    nc.sync.dma_start(imgWT[:], ins["imgWT"][:, :])
    imgb = mk(sm, [3, 1], "imgb")
    nc.sync.dma_start(imgb[:], ins["imgb"][:, :])

    # out state: starts as seed
    out_tag = "out_a"
    out_t = mk(big, [F, 256], "out_a")
    nc.sync.dma_start(out_t[:], ins["seed"][:, :])

    v121 = [0.125, 0.25, 0.125]   # [1,2,1]/8 vertical taps for sobel_x
    vdif = [-0.125, 0.0, 0.125]   # [-1,0,1]/8 vertical taps for sobel_y

    pa_next = phase_a_stream(0)
    phase_a_cc(*pa_next)
    for l in range(L):
        if l + 1 < L:
            pa_next = phase_a_stream(l + 1)
        s = SIZES[l]
        hw = HWS[l]
        nch = max(1, hw // 512)
        cn0 = min(512, hw)

        # -------- generated weights for this layer
        bt_k = load_w(l, "bt", 1024, 64)
        c2w_w = load_w(l, "c2w", 64, 32, keep_f32=True)[0]
        w2cA_w = load_w(l, "w2cA", 32, 128)[0]
        w2cBc, w2cBs = load_w(l, "w2cB", 128, 128, split64=True)
        g2cA, g2cB = load_w(l, "g2c", 128, 128, split64=True)
        skip_k = load_w(l, "skip", 256, 64)
        c1_k = load_w(l, "c1", 256, 128)
        c2_w = load_w(l, "c2", 128, 128)[0]
        c3_w = load_w(l, "c3", 128, 64)[0]
        bt_b = load_b(l, "bt", 64)
        c2w_b = load_b(l, "c2w", 32)
        skip_b = load_b(l, "skip", 64)
        c1_b = load_b(l, "c1", 128)
        c2_b = load_b(l, "c2", 128)
        c3_b = load_b(l, "c3", 64)

        bt_b_cos = mk(gw, [F, 1], "bt_b_cos")
        nc.vector.tensor_scalar_add(bt_b_cos[:], bt_b[:], PI_2)
        # per-channel constants cos(bias)/sin(bias): subtracted from cos/sin
        # before bf16 rounding (instance norm cancels channel constants
        # exactly, and the centered values are small enough for bf16)
        cos_b = mk(gw, [F, 1], "cos_b")
        nc.scalar.activation(cos_b[:], bt_b_cos[:], ACTF.Sin)
        sin_b = mk(gw, [F, 1], "sin_b")
        nc.scalar.activation(sin_b[:], bt_b[:], ACTF.Sin)
        bias_sc = mk(gw, [F, 1], "bias_sc")
        nc.vector.tensor_add(bias_sc[:], skip_b[:], c3_b[:])
        nc.vector.tensor_mul(bias_sc[:], bias_sc[:], leak64[:])

        # sobel folded: grads_enc = (A/8) @ V121(t_hd) + (B/8) @ Vdiff(t_h1)
        wA = mk(gw, [64, 128], "wA", bf16)
        nc.vector.tensor_scalar(wA[:], g2cA[:], 0.125, None, ALU.mult)
        wB = mk(gw, [64, 128], "wB", bf16)
        nc.vector.tensor_scalar(wB[:], g2cB[:], 0.125, None, ALU.mult)

        # cell wave
        cw_t = mk(big, [CPE, hw], "cw", bf16)
        nc.sync.dma_start(cw_t[:], ins[f"cw{l}"][:, :])

        # -------- surf -> cos/sin tiles (E1 folded into w2cB weights)
        cos_t = mk(big, [F, hw], "cos_t", bf16)
        sin_t = mk(big, [F, hw], "sin_t", bf16)
        wb_in = ins[f"wb{l}"]
        g0 = 0
        while g0 < hw:
            gn = min(1024, hw - g0)
            nsub = (gn + 511) // 512
            pss = [mk(psp, [F, 512], "we", bufs=2) for _ in range(nsub)]
            for kc in range(8):
                wbt = mk(wbp, [128, 1024], "wbt", bf16)
                nc.sync.dma_start(wbt[:, :gn],
                                  wb_in[kc * 128:(kc + 1) * 128, g0:g0 + gn])
                for u in range(nsub):
                    un = min(512, gn - u * 512)
                    mmr(pss[u][:, :un], bt_k[kc][:],
                        wbt[:, u * 512:u * 512 + un], kc == 0, kc == 7)
            for u in range(nsub):
                un = min(512, gn - u * 512)
                lo = g0 + u * 512
                tt = mk(ab, [F, 512], "trigtmp")
                nc.scalar.activation(tt[:, :un], pss[u][:, :un],
                                     ACTF.Sin, bias=bt_b_cos[:])
                nc.vector.tensor_scalar(cos_t[:, lo:lo + un], tt[:, :un],
                                        cos_b[:], None, ALU.subtract)
                tt2 = mk(ab, [F, 512], "trigtmp")
                nc.scalar.activation(tt2[:, :un], pss[u][:, :un],
                                     ACTF.Sin, bias=bt_b[:])
                nc.vector.tensor_scalar(sin_t[:, lo:lo + un], tt2[:, :un],
                                        sin_b[:], None, ALU.subtract)
            g0 += gn

        out3 = out_t[:].rearrange("p (h w) -> p h w", h=s)

        if l == L - 1:
            img_sb = mk(big, [3, hw], "cw")
            imgWT_bf = mk(sm, [F, 3], "imgWT_bf", bf16)
            nc.vector.tensor_copy(imgWT_bf[:], imgWT[:])

        for call in range(2):
            # ---- wave_c coefficients (mean folded: 1/hw pre-scaled into c2w)
            so = mk(smc, [F, 1], "so")
            nc.vector.tensor_reduce(so[:], out3, axis=AX.XY, op=ALU.add)
            ps_c = mk(psp, [CPE, 1], "c2")
            mm32(ps_c[:], c2w_w[:], so[:], True, True)
            coef = mk(smc, [CPE, 1], "coef")
            nc.scalar.activation(coef[:], ps_c[:], ACTF.Identity, bias=c2w_b[:])
            w2cA_eff = mk(gw, [CPE, 128], "w2cAe", bf16)
            nc.vector.tensor_scalar(w2cA_eff[:], w2cA_w[:], coef[:], None,
                                    ALU.mult)

            # ---- t buffers: t_hd = H-diff(out), t_h1 = H-121(out); bf16 for
            # the PE, with an f32 scratch so no op mixes input dtypes
            t_hd = mk(big, [F, hw], "big1", bf16)
            t_h1 = mk(big, [F, hw], "big2", bf16)
            t1f = mk(big, [F, hw], "weraw")
            d3 = t_hd[:].rearrange("p (h w) -> p h w", h=s)
            o3 = t_h1[:].rearrange("p (h w) -> p h w", h=s)
            f3 = t1f[:].rearrange("p (h w) -> p h w", h=s)
            nc.vector.tensor_sub(d3[:, :, 1:s - 1], out3[:, :, 2:s],
                                 out3[:, :, 0:s - 2])
            nc.vector.tensor_copy(d3[:, :, 0:1], out3[:, :, 1:2])
            nc.vector.tensor_scalar(d3[:, :, s - 1:s],
                                    out3[:, :, s - 2:s - 1], -1.0, None,
                                    ALU.mult)
            nc.vector.tensor_add(f3[:, :, 1:s - 1], out3[:, :, 2:s],
                                 out3[:, :, 0:s - 2])
            nc.vector.tensor_copy(f3[:, :, 0:1], out3[:, :, 1:2])
            nc.vector.tensor_copy(f3[:, :, s - 1:s], out3[:, :, s - 2:s - 1])
            nc.vector.scalar_tensor_tensor(o3[:, :, :], out3[:, :, :], 2.0,
                                           f3[:, :, :], ALU.mult, ALU.add)

            # vertical sobel passes on DVE (bf16 4x mode):
            # u1 = V121(t_hd), u2 = Vdiff(t_h1), zero-padded
            u1 = mk(big, [F, hw], "u1", bf16)
            u2 = mk(big, [F, hw], "u2", bf16)
            u13 = u1[:].rearrange("p (h w) -> p h w", h=s)
            u23 = u2[:].rearrange("p (h w) -> p h w", h=s)
            nc.vector.tensor_add(u13[:, 1:s - 1, :], d3[:, 0:s - 2, :],
                                 d3[:, 2:s, :])
            nc.vector.tensor_copy(u13[:, 0:1, :], d3[:, 1:2, :])
            nc.vector.tensor_copy(u13[:, s - 1:s, :], d3[:, s - 2:s - 1, :])
            nc.vector.scalar_tensor_tensor(u13[:, :, :], d3[:, :, :], 2.0,
                                           u13[:, :, :], ALU.mult, ALU.add)
            nc.vector.tensor_sub(u23[:, 1:s - 1, :], o3[:, 2:s, :],
                                 o3[:, 0:s - 2, :])
            nc.vector.tensor_copy(u23[:, 0:1, :], o3[:, 1:2, :])
            nc.vector.tensor_scalar(u23[:, s - 1:s, :], o3[:, s - 2:s - 1, :],
                                    -1.0, None, ALU.mult)

            # ---- wave_enc / grads_enc raw + stats
            we_raw = mk(big, [128, hw], "weraw")
            ge_raw = mk(big, [128, hw], "geraw")
            we_st = mk(smc, [128, 6 * nch], "west")
            ge_st = mk(smc, [128, 6 * nch], "gest")
            for n in range(nch):
                c0 = n * cn0
                cn = cn0
                ps_we = mk(psp, [128, 512], "we", bufs=2)
                mmr(ps_we[:, :cn], w2cA_eff[:], cw_t[:, c0:c0 + cn], True, False)
                mmr(ps_we[:, :cn], w2cBc[:], cos_t[:, c0:c0 + cn], False, False)
                mmr(ps_we[:, :cn], w2cBs[:], sin_t[:, c0:c0 + cn], False, True)
                ps_ge = mk(psp, [128, 512], "ge")
                mmr(ps_ge[:, :cn], wA[:], u1[:, c0:c0 + cn], True, False)
                mmr(ps_ge[:, :cn], wB[:], u2[:, c0:c0 + cn], False, True)
                nc.vector.bn_stats(we_st[:, n * 6:(n + 1) * 6], ps_we[:, :cn])
                nc.vector.bn_stats(ge_st[:, n * 6:(n + 1) * 6], ps_ge[:, :cn])
                nc.scalar.copy(we_raw[:, c0:c0 + cn], ps_we[:, :cn])
                nc.vector.tensor_copy(ge_raw[:, c0:c0 + cn], ps_ge[:, :cn])

            # ---- instance-norm scale/bias
            def inorm_apply(st, raw, nm):
                mv = mk(smc, [128, 2], f"mv{nm}")
                nc.vector.bn_aggr(mv[:], st[:])
                ve = mk(smc, [128, 1], f"ve{nm}")
                nc.vector.tensor_scalar(ve[:], mv[:, 1:2], 1e-5, None, ALU.add)
                nc.vector.reciprocal(ve[:], ve[:])
                rs = mk(smc, [128, 1], f"rs{nm}")
                nc.scalar.sqrt(rs[:], ve[:])
                nb = mk(smc, [128, 1], f"nb{nm}")
                nc.vector.tensor_mul(nb[:], mv[:, 0:1], rs[:])
                nc.vector.tensor_scalar(nb[:], nb[:], -1.0, None, ALU.mult)
                normed = mk(big, [128, hw], f"{nm}_n", bf16)
                nc.scalar.activation(normed[:, :], raw[:, :], ACTF.Identity,
                                     bias=nb[:], scale=rs[:])
                return normed

            we_n = inorm_apply(we_st, we_raw, "we")
            ge_n = inorm_apply(ge_st, ge_raw, "ge")

            # ---- main branch: skip + c1/lrelu/c2/lrelu/c3, residual update
            for n in range(nch):
                c0 = n * cn0
                cn = cn0
                ps_s = mk(psp, [F, 512], "sc", bufs=2)
                mmr(ps_s[:, :cn], skip_k[0][:], ge_n[:, c0:c0 + cn], True,
                    False)
                mmr(ps_s[:, :cn], skip_k[1][:], we_n[:, c0:c0 + cn], False,
                    False)
                ps_1 = mk(psp, [128, 512], "c1")
                mmr(ps_1[:, :cn], c1_k[0][:], ge_n[:, c0:c0 + cn], True, False)
                mmr(ps_1[:, :cn], c1_k[1][:], we_n[:, c0:c0 + cn], False, True)
                a1 = mk(ab, [128, 512], "a1", bf16)
                nc.scalar.activation(a1[:, :cn], ps_1[:, :cn], ACTF.Identity,
                                     bias=c1_b[:])
                nc.vector.scalar_tensor_tensor(a1[:, :cn], a1[:, :cn], 0.2,
                                               a1[:, :cn], ALU.mult, ALU.max)
                ps_2 = mk(psp, [128, 512], "c2")
                mmr(ps_2[:, :cn], c2_w[:], a1[:, :cn], True, True)
                a2 = mk(ab, [128, 512], "a2", bf16)
                nc.scalar.activation(a2[:, :cn], ps_2[:, :cn], ACTF.Identity,
                                     bias=c2_b[:])
                nc.vector.scalar_tensor_tensor(a2[:, :cn], a2[:, :cn], 0.2,
                                               a2[:, :cn], ALU.mult, ALU.max)
                mmr(ps_s[:, :cn], c3_w[:], a2[:, :cn], False, True)
                tn = mk(ab, [F, 512], "tn")
                nc.scalar.activation(tn[:, :cn], ps_s[:, :cn], ACTF.Identity,
                                     bias=bias_sc[:], scale=leak64[:])
                nc.vector.tensor_add(out_t[:, c0:c0 + cn], out_t[:, c0:c0 + cn],
                                     tn[:, :cn])
                if l == L - 1 and call == 1:
                    ob = mk(ab, [F, 512], "ob", bf16)
                    nc.scalar.copy(ob[:, :cn], out_t[:, c0:c0 + cn])
                    ps_i = mk(psp, [3, 512], "c2")
                    mmr(ps_i[:, :cn], imgWT_bf[:], ob[:, :cn], True, True)
                    nc.scalar.activation(img_sb[:, c0:c0 + cn], ps_i[:, :cn],
                                         ACTF.Identity, bias=imgb[:])
                    nc.vector.tensor_scalar(img_sb[:, c0:c0 + cn],
                                            img_sb[:, c0:c0 + cn], -1.0, 1.0,
                                            ALU.max, ALU.min)

        if l + 1 < L:
            phase_a_cc(*pa_next)

        # -------- layer transition: bilinear 2x upsample + gauss, or image out
        if l < L - 1:
            s2 = 2 * s
            upv = mk(big, [F, s2 * s], "weraw")
            v3 = upv[:].rearrange("p (h w) -> p h w", h=s2)
            # vertical: rows carry 1/0.75 scale (folded into final constant)
            nc.vector.scalar_tensor_tensor(
                v3[:, 2:s2:2, :], out3[:, 0:s - 1, :], 1.0 / 3.0,
                out3[:, 1:s, :], ALU.mult, ALU.add)
            nc.vector.tensor_scalar(v3[:, 0:1, :], out3[:, 0:1, :], 4.0 / 3.0,
                                    None, ALU.mult)
            nc.vector.scalar_tensor_tensor(
                v3[:, 1:s2 - 1:2, :], out3[:, 1:s, :], 1.0 / 3.0,
                out3[:, 0:s - 1, :], ALU.mult, ALU.add)
            nc.vector.tensor_scalar(v3[:, s2 - 1:s2, :], out3[:, s - 1:s, :],
                                    4.0 / 3.0, None, ALU.mult)
            uph = mk(big, [F, s2 * s2], "geraw")
            h3 = uph[:].rearrange("p (h w) -> p h w", h=s2)
            nc.vector.scalar_tensor_tensor(
                h3[:, :, 2:s2:2], v3[:, :, 0:s - 1], 1.0 / 3.0, v3[:, :, 1:s],
                ALU.mult, ALU.add)
            nc.vector.tensor_scalar(h3[:, :, 0:1], v3[:, :, 0:1], 4.0 / 3.0,
                                    None, ALU.mult)
            nc.vector.scalar_tensor_tensor(
                h3[:, :, 1:s2 - 1:2], v3[:, :, 1:s], 1.0 / 3.0,
                v3[:, :, 0:s - 1], ALU.mult, ALU.add)
            nc.vector.tensor_scalar(h3[:, :, s2 - 1:s2], v3[:, :, s - 1:s],
                                    4.0 / 3.0, None, ALU.mult)
            # gauss 3x3 (zero pad), separable; final scale folds everything
            gu = mk(big, [F, s2 * s2], "weraw")
            u3 = gu[:].rearrange("p (h w) -> p h w", h=s2)
            nc.vector.tensor_add(u3[:, 1:s2 - 1, :], h3[:, 0:s2 - 2, :],
                                 h3[:, 2:s2, :])
            nc.vector.tensor_copy(u3[:, 0:1, :], h3[:, 1:2, :])
            nc.vector.tensor_copy(u3[:, s2 - 1:s2, :], h3[:, s2 - 2:s2 - 1, :])
            gt = mk(big, [F, s2 * s2], out_tag)
            t3g = gt[:].rearrange("p (h w) -> p h w", h=s2)
            nc.vector.scalar_tensor_tensor(t3g[:, :, :], h3[:, :, :], GRBA,
                                           u3[:, :, :], ALU.mult, ALU.add)
            gu2 = mk(big, [F, s2 * s2], "geraw")
            u23 = gu2[:].rearrange("p (h w) -> p h w", h=s2)
            nc.vector.tensor_add(u23[:, :, 1:s2 - 1], t3g[:, :, 0:s2 - 2],
                                 t3g[:, :, 2:s2])
            nc.vector.tensor_copy(u23[:, :, 0:1], t3g[:, :, 1:2])
            nc.vector.tensor_copy(u23[:, :, s2 - 1:s2], t3g[:, :, s2 - 2:s2 - 1])
            nc.vector.scalar_tensor_tensor(u23[:, :, :], t3g[:, :, :], GRBA,
                                           u23[:, :, :], ALU.mult, ALU.add)
            newout = mk(big, [F, s2 * s2], "out_b" if l == 0 else "out_a")
            nc.scalar.activation(newout[:], gu2[:], ACTF.Copy,
                                 scale=UPGAUSS_C)
            out_t = newout
            out3 = out_t[:].rearrange("p (h w) -> p h w", h=s2)
        else:
            nc.sync.dma_start(out_img[:, :], img_sb[:])

    for p in reversed(ctxs):
        p.__exit__(None, None, None)


_MODULE_CACHE = {}


def build_module():
    if "nc" in _MODULE_CACHE:
        return _MODULE_CACHE["nc"]
    from concourse import bacc, mybir
    import concourse.tile as tile

    nc = bacc.Bacc("TRN2", target_bir_lowering=False, debug=False,
                   num_devices=NCORE)
    f32 = mybir.dt.float32
    dts = {"f32": mybir.dt.float32, "bf16": mybir.dt.bfloat16}
    ins = {}
    for name, shape, dt in INPUT_SHAPES:
        ins[name] = nc.dram_tensor(name, list(shape), dts[dt],
                                   kind="ExternalInput").ap()
    out_img = nc.dram_tensor("img", [3, HWS[-1]], f32,
                             kind="ExternalOutput").ap()
    with tile.TileContext(nc) as tc:
        build_kernel(tc, ins, out_img)
    nc.compile()
    _MODULE_CACHE["nc"] = nc
    return nc


# ---------------------------------------------------------------- host prep
def host_prep(inputs):
    """Full (unsharded) numpy inputs -> per-core in_maps."""
    lat = np.asarray(inputs["lat"], np.float32)
    leak = float(np.clip(np.asarray(inputs["leak_factor"]), 0.001, 1000.0))
    seed = np.asarray(inputs["seed"], np.float32)[0].reshape(F, 256)
    frac_W = np.asarray(inputs["frac_W"], np.float32)
    frac_b = np.asarray(inputs["frac_b"], np.float32)

    def T(W, fin, fout):
        W = np.asarray(W, np.float32)
        return W.reshape(L, LAT, fout, fin).transpose(0, 1, 3, 2)

    bt_T = T(inputs["bt_W"], 1024, F)
    c2w_T = T(inputs["c2w_W"], F, CPE).copy()
    for l in range(L):
        c2w_T[l] *= 1.0 / HWS[l]
    w2c_T = T(inputs["w2c_W"], CPE + 2 * F, 2 * F)
    w2cA_T = w2c_T[:, :, :CPE, :]
    w2cB_T = np.ascontiguousarray(w2c_T[:, :, CPE:, :]) * np.float32(E1)
    g2c_T = T(inputs["g2c_W"], 2 * F, 2 * F)
    skip_T = T(inputs["cc_skip_W"], 4 * F, F)
    c1_T = T(inputs["cc_c1_W"], 4 * F, 2 * F)
    c2_T = T(inputs["cc_c2_W"], 2 * F, 2 * F)
    c3_T = T(inputs["cc_c3_W"], 2 * F, F)
    convT = {"bt": bt_T, "c2w": c2w_T, "w2cA": w2cA_T, "w2cB": w2cB_T,
             "g2c": g2c_T, "skip": skip_T, "c1": c1_T, "c2": c2_T, "c3": c3_T}

    bias_blk = np.concatenate(
        [np.asarray(inputs[k], np.float32) for k in
         ("bt_b", "c2w_b", "cc_skip_b", "cc_c1_b", "cc_c2_b", "cc_c3_b")],
        axis=2)  # (L, LAT, 480)

    import ml_dtypes

    bf16 = ml_dtypes.bfloat16
    common = {
        "latT": np.ascontiguousarray(lat.T),
        "fracW": frac_W,
        "fracb": frac_b,
        "leak": np.full((1, 1), leak, np.float32),
        "seed": np.ascontiguousarray(seed),
        "imgWT": np.ascontiguousarray(np.asarray(inputs["img_W"],
                                                 np.float32).T),
        "imgb": np.asarray(inputs["img_b"], np.float32).reshape(3, 1),
    }
    for l in range(L):
        common[f"wb{l}"] = np.ascontiguousarray(
            np.asarray(inputs[f"wave_bias{l}"], np.float32)[0].reshape(
                F * 16, HWS[l])).astype(bf16)
        common[f"cw{l}"] = pos_enc_np(SIZES[l]).astype(bf16)

    in_maps = []
    for j in range(NCORE):
        blocks = []
        for name, fin, fout in CONVS:
            r = fin // NCORE
            blocks.append(convT[name][:, :, j * r:(j + 1) * r, :].reshape(
                L, LAT, r * fout))
        blocks.append(bias_blk)
        wws = np.concatenate(blocks, axis=2).astype(bf16)
        assert wws.shape == (L, LAT, SCOLS_T), wws.shape
        m = dict(common)
        m["wws"] = wws
        in_maps.append(m)
    return in_maps


def kernel(**inputs):
    from concourse.bass_utils import run_bass_kernel_spmd

    nc = build_module()
    in_maps = host_prep(inputs)
    res = run_bass_kernel_spmd(nc, in_maps, list(range(NCORE)))
    imgs = [res.results[b]["img"].reshape(3, SIZES[-1], SIZES[-1])
            for b in range(B)]
    return np.stack(imgs).astype(np.float32)


if __name__ == "__main__":
    nc = build_module()
    print("module built ok; instructions:",
          sum(len(bb.instructions) for bb in nc.main_func.blocks))



# revision 16
# speedup vs baseline: 1.2059x; 1.2059x over previous
"""Trainium2 Bass kernel for nn_Decoder_72911364816952 (v2).

Strategy (8 NeuronCores, memory-bound regime):
  Phase A (all cores): stream the dynamic-weight generator matrices Ww
    (bf16, 1/8 column-shard per core, host-sliced) through the PE with the
    lat_new vectors stationary (M=8 = all destination ranks at once).
    Columns are split into an EARLY block (bt/c2w/w2c/g2c + biases) and a
    LATE block (skip/c1/c2/c3) with separate AllToAlls per layer so phase B
    can start as soon as the early weights arrive.
  Phase B (each core, one sample): surf/sin/cos wave path, Sobel folded
    into the g2c matmuls, instance norm FOLDED INTO the skip/c1 weights
    (scale lhsT rows by rs, bias fixup via N=1 matmuls), fused
    bilinear-2x-upsample+gauss as a 3-tap polyphase filter, elementwise
    work spread across Vector/GpSimd/Scalar engines, channel sums for the
    wave coefficient and instance norms picked up for free via accum_out.
"""
import math
import sys

sys.path.insert(0, "/opt/trn_rl_repo")

import numpy as np

# ---------------------------------------------------------------- constants
L = 3
LAT = 256
F = 64
CPE = 32
B = 4
NCORE = 8
SIZES = [16, 32, 64]
HWS = [s * s for s in SIZES]
K_FREQ = 8

# EARLY block per-core column layout (offsets in elements)
OFF_BT, N_BT = 0, 8192          # fin-shard: rank r = fin rows r*128.., (il=128, o=64)
OFF_C2W, N_C2W = 8192, 256      # fout-shard fo=4 (p=64, fo=4)
OFF_W2CA, N_W2CA = 8448, 512    # fout-shard fo=16 (p=32, fo=16)
OFF_W2CB, N_W2CB = 8960, 2048   # fout-shard fo=16, 2 chunks (c=2, p=64, fo=16)
OFF_G2C, N_G2C = 11008, 2048    # fout-shard fo=16, 2 chunks (c=2, p=64, fo=16)
OFF_BIAS, N_BIAS = 13056, 128   # bias column j from rank j
SCOLS_E = 13184
# LATE block
OFF_SKIP, N_SKIP = 0, 2048      # fo=8, 2 chunks (c=2, p=128, fo=8)
OFF_C1, N_C1 = 2048, 4096       # fo=16, 2 chunks (c=2, p=128, fo=16)
OFF_C2, N_C2 = 6144, 2048       # fo=16, 1 chunk
OFF_C3, N_C3 = 8192, 1024      # fo=8, 1 chunk
SCOLS_L = 9216

E1 = math.e
_ga_raw = math.exp(-0.5)
GA = _ga_raw / (1.0 + 2.0 * _ga_raw)   # normalized 1D gauss edge tap (a)
GB = 1.0 / (1.0 + 2.0 * _ga_raw)       # center tap (b)
# composite bilinear-2x + gauss 3-tap polyphase coefficients
UC0 = 0.75 * GA + 0.25 * GB
UC1 = GA + 0.75 * GB
UC2 = 0.25 * GA
UA = UC0 / UC2
UBC = UC1 / UC2
UE00 = (GB + 0.75 * GA) / UC2
UE1C = (1.25 * GA + 0.75 * GB) / UC2
UE2C = (0.75 * GA + GB) / UC2
PI_2 = math.pi / 2.0


def pos_enc_np(size):
    p = np.arange(size, dtype=np.float32)
    feats = []
    for k in range(K_FREQ):
        ang = p * (2.0 ** k) * (2.0 * np.pi / size)
        s, c = np.sin(ang).astype(np.float32), np.cos(ang).astype(np.float32)
        feats += [np.broadcast_to(s[:, None], (size, size)),
                  np.broadcast_to(c[:, None], (size, size)),
                  np.broadcast_to(s[None, :], (size, size)),
                  np.broadcast_to(c[None, :], (size, size))]
    return np.ascontiguousarray(np.stack(feats).reshape(4 * K_FREQ, size * size))


INPUT_SHAPES = [
    ("wwsE", (L, LAT, SCOLS_E), "bf16"),
    ("wwsL", (L, LAT, SCOLS_L), "bf16"),
    ("latT", (LAT, B), "f32"),
    ("fracW", (L, LAT, LAT), "f32"),
    ("fracb", (L, LAT), "f32"),
    ("leak", (1, 1), "f32"),
    ("seed", (F, 256), "f32"),
    ("wb0", (1024, HWS[0]), "bf16"),
    ("wb1", (1024, HWS[1]), "bf16"),
    ("wb2", (1024, HWS[2]), "bf16"),
    ("cw0", (CPE, HWS[0]), "bf16"),
    ("cw1", (CPE, HWS[1]), "bf16"),
    ("cw2", (CPE, HWS[2]), "bf16"),
    ("imgWT", (F, 3), "f32"),
    ("imgb", (3, 1), "f32"),
]


# ---------------------------------------------------------------- device code
def build_kernel(tc, ins, out_img):
    import concourse.bass as bass
    from concourse import mybir

    nc = tc.nc
    f32 = mybir.dt.float32
    bf16 = mybir.dt.bfloat16
    ALU = mybir.AluOpType
    ACTF = mybir.ActivationFunctionType
    AX = mybir.AxisListType
    ctxs = []

    def pool(name, bufs, space="SBUF"):
        p = tc.tile_pool(name=name, bufs=bufs, space=space)
        ctxs.append(p)
        return p.__enter__()

    def mk(pool_, shape, tag, dt=None, bufs=None):
        return pool_.tile(shape, dt or f32, name=tag, tag=tag, bufs=bufs)

    dram = pool("dram", 1, "DRAM")
    psp = pool("psp", 1, "PSUM")
    big = pool("big", 1)
    gw = pool("gw", 2)
    ab = pool("ab", 3)
    sm = pool("sm", 1)
    smc = pool("smc", 2)
    pha = pool("pha", 2)
    stg = pool("stg", 2)
    wbp = pool("wbp", 3)

    def mm(ps, lhsT, rhs, start, stop):
        nc.tensor.matmul(ps, lhsT, rhs, start=start, stop=stop)

    # ---------------- setup: latT, leak, lat_new8 per layer
    latT = []
    for kc in range(2):
        t = mk(sm, [128, B], f"latT{kc}")
        nc.sync.dma_start(t[:], ins["latT"][kc * 128:(kc + 1) * 128, :])
        latT.append(t)

    leak_sb = mk(sm, [1, 1], "leak")
    nc.sync.dma_start(leak_sb[:], ins["leak"][:, :])
    ones64 = mk(sm, [1, F], "ones64")
    nc.vector.memset(ones64[:], 1.0)
    ps_l = mk(psp, [F, 1], "mini", bufs=1)
    mm(ps_l[:], ones64[:], leak_sb[:], True, True)
    leak64 = mk(sm, [F, 1], "leak64")
    nc.scalar.copy(leak64[:], ps_l[:])

    lat8 = []  # [l][kc] -> (128, 8) bf16 stationary tiles
    for l in range(L):
        fb = []
        for kc in range(2):
            t = mk(sm, [128, 1], f"fracb{l}{kc}")
            nc.sync.dma_start(
                t[:], ins["fracb"][l, kc * 128:(kc + 1) * 128].rearrange(
                    "(p u) -> p u", u=1))
            fb.append(t)
        lnT = []
        for mc in range(2):
            ps = mk(psp, [128, B], "mini", bufs=1)
            for kc in range(2):
                fw = mk(sm, [128, 128], "fwtile", bufs=2)
                nc.sync.dma_start(
                    fw[:], ins["fracW"][l, kc * 128:(kc + 1) * 128,
                                        mc * 128:(mc + 1) * 128])
                mm(ps[:], fw[:], latT[kc][:], kc == 0, kc == 1)
            t = mk(sm, [128, 8], f"latnew{l}{mc}", bf16)
            nc.scalar.activation(t[:, 0:4], ps[:], ACTF.Identity, bias=fb[mc][:])
            nc.scalar.activation(t[:, 4:8], ps[:], ACTF.Identity, bias=fb[mc][:])
            lnT.append(t)
        lat8.append(lnT)

    imgWT = mk(sm, [F, 3], "imgWT")
    nc.sync.dma_start(imgWT[:], ins["imgWT"][:, :])
    imgWT_bf = mk(sm, [F, 3], "imgWT_bf", bf16)
    nc.vector.tensor_copy(imgWT_bf[:], imgWT[:])
    imgb = mk(sm, [3, 1], "imgb")
    nc.sync.dma_start(imgb[:], ins["imgb"][:, :])

    # out state: starts as seed
    out_t = mk(big, [F, 256], "out_a")
    nc.sync.dma_start(out_t[:], ins["seed"][:, :])

    # ---------------- phase A task machinery
    gatE = {}
    gatL = {}
    _di = [0]

    def pa_chunk(wws, l, contrib, c0, dc):
        rhs0 = mk(pha, [128, 2048], "rhs0", bf16)
        rhs1 = mk(pha, [128, 2048], "rhs1", bf16)
        nc.sync.dma_start(rhs0[:, :dc], wws[l, 0:128, c0:c0 + dc])
        nc.sync.dma_start(rhs1[:, :dc], wws[l, 128:256, c0:c0 + dc])
        stage = mk(stg, [8, 2048], "stage", bf16)
        n0 = 0
        while n0 < dc:
            nn = min(512, dc - n0)
            ps = mk(psp, [8, 512], "pa", bufs=3)
            mm(ps[:, :nn], lat8[l][0][:], rhs0[:, n0:n0 + nn], True, False)
            mm(ps[:, :nn], lat8[l][1][:], rhs1[:, n0:n0 + nn], False, True)
            if _di[0] % 2 == 0:
                nc.scalar.copy(stage[:, n0:n0 + nn], ps[:, :nn])
            else:
                nc.vector.tensor_copy(stage[:, n0:n0 + nn], ps[:, :nn])
            _di[0] += 1
            n0 += nn
        nc.scalar.dma_start(contrib[0:8, c0:c0 + dc], stage[:, :dc])

    def pa_a2a(contrib, gathered):
        nc.gpsimd.collective_compute(
            "AllToAll", mybir.AluOpType.bypass,
            replica_groups=[list(range(NCORE))],
            ins=[contrib.opt()],
            outs=[gathered.opt()],
        )

    pa_tasks = []

    def build_layer_tasks(l, block):
        wws = ins["wwsE"] if block == "E" else ins["wwsL"]
        scols = SCOLS_E if block == "E" else SCOLS_L
        contrib = mk(dram, [NCORE, scols], f"contrib{block}{l}", bf16)
        gathered = mk(dram, [NCORE, scols], f"a2a{block}{l}", bf16)
        (gatE if block == "E" else gatL)[l] = gathered
        c0 = 0
        while c0 < scols:
            dc = min(2048, scols - c0)
            pa_tasks.append(
                lambda wws=wws, l=l, contrib=contrib, c0=c0, dc=dc:
                pa_chunk(wws, l, contrib, c0, dc))
            c0 += dc
        pa_tasks.append(lambda c=contrib, g=gathered: pa_a2a(c, g))

    for l in range(L):
        build_layer_tasks(l, "E")
        build_layer_tasks(l, "L")

    def pump(n):
        for _ in range(n):
            if pa_tasks:
                pa_tasks.pop(0)()

    # ---------------- phase B helpers
    def ld(dst, src):
        nc.gpsimd.dma_start(dst, src)

    def load_early(l):
        gE = gatE[l]
        w = {}
        btk = mk(gw, [128, 512], "btk", bf16)
        ld(btk[:].rearrange("p (r o) -> p r o", o=64),
           gE[0:8, OFF_BT:OFF_BT + N_BT].rearrange("r (il o) -> il r o", o=64))
        c2w_w = mk(gw, [64, 32], "c2ww", bf16)
        ld(c2w_w[:].rearrange("p (r fo) -> p r fo", fo=4),
           gE[0:8, OFF_C2W:OFF_C2W + N_C2W].rearrange("r (p fo) -> p r fo", fo=4))
        w2ca = mk(gw, [96, 128], "w2ca", bf16)
        ld(w2ca[64:96, :].rearrange("p (r fo) -> p r fo", fo=16),
           gE[0:8, OFF_W2CA:OFF_W2CA + N_W2CA].rearrange(
               "r (p fo) -> p r fo", fo=16))
        weT = mk(gw, [96, 128], "weT", bf16)
        ld(weT[0:64, :].rearrange("p (r fo) -> p r fo", fo=16),
           gE[0:8, OFF_W2CB:OFF_W2CB + 1024].rearrange(
               "r (p fo) -> p r fo", fo=16))
        w2cbs = mk(gw, [64, 128], "w2cbs", bf16)
        ld(w2cbs[:].rearrange("p (r fo) -> p r fo", fo=16),
           gE[0:8, OFF_W2CB + 1024:OFF_W2CB + 2048].rearrange(
               "r (p fo) -> p r fo", fo=16))
        g2ca = mk(gw, [64, 128], "g2ca", bf16)
        ld(g2ca[:].rearrange("p (r fo) -> p r fo", fo=16),
           gE[0:8, OFF_G2C:OFF_G2C + 1024].rearrange(
               "r (p fo) -> p r fo", fo=16))
        g2cb = mk(gw, [64, 128], "g2cb", bf16)
        ld(g2cb[:].rearrange("p (r fo) -> p r fo", fo=16),
           gE[0:8, OFF_G2C + 1024:OFF_G2C + 2048].rearrange(
               "r (p fo) -> p r fo", fo=16))
        b8h = mk(gw, [128, 8], "b8h", bf16)
        ld(b8h[:], gE[0:8, OFF_BIAS:OFF_BIAS + N_BIAS].rearrange("r p -> p r"))
        b8 = mk(gw, [128, 8], "b8")
        nc.vector.tensor_copy(b8[:], b8h[:])
        wA = mk(gw, [64, 128], "wA", bf16)
        nc.vector.tensor_scalar(wA[:], g2ca[:], 0.125, None, ALU.mult)
        wB = mk(gw, [64, 128], "wB", bf16)
        nc.gpsimd.tensor_scalar(wB[:], g2cb[:], 0.125, None, ALU.mult)
        btb_cos = mk(gw, [64, 1], "btbc")
        nc.vector.tensor_scalar(btb_cos[:], b8[0:64, 2:3], PI_2, None, ALU.add)
        cos_b = mk(gw, [64, 1], "cos_b")
        nc.scalar.activation(cos_b[:], btb_cos[:], ACTF.Sin)
        sin_b = mk(gw, [64, 1], "sin_b")
        nc.scalar.activation(sin_b[:], b8[0:64, 2:3], ACTF.Sin)
        skipc3 = mk(gw, [64, 1], "skc3")
        nc.vector.tensor_add(skipc3[:], b8[0:64, 4:5], b8[0:64, 5:6])
        w.update(btk=btk, c2w_w=c2w_w, w2ca=w2ca, weT=weT, w2cbs=w2cbs,
                 wA=wA, wB=wB, b8=b8, btb_cos=btb_cos, cos_b=cos_b,
                 sin_b=sin_b, skipc3=skipc3)
        return w

    def load_late(l, w):
        gL = gatL[l]
        skip_k = mk(gw, [128, 128], "skipk", bf16)
        for c in range(2):
            ld(skip_k[:, c * 64:(c + 1) * 64].rearrange(
                   "p (r fo) -> p r fo", fo=8),
               gL[0:8, OFF_SKIP + c * 1024:OFF_SKIP + (c + 1) * 1024].rearrange(
                   "r (p fo) -> p r fo", fo=8))
        c1_k = mk(gw, [128, 256], "c1k", bf16)
        for c in range(2):
            ld(c1_k[:, c * 128:(c + 1) * 128].rearrange(
                   "p (r fo) -> p r fo", fo=16),
               gL[0:8, OFF_C1 + c * 2048:OFF_C1 + (c + 1) * 2048].rearrange(
                   "r (p fo) -> p r fo", fo=16))
        c2_w = mk(gw, [128, 128], "c2w_", bf16)
        ld(c2_w[:].rearrange("p (r fo) -> p r fo", fo=16),
           gL[0:8, OFF_C2:OFF_C2 + N_C2].rearrange("r (p fo) -> p r fo", fo=16))
        c3_w = mk(gw, [128, 64], "c3w", bf16)
        ld(c3_w[:].rearrange("p (r fo) -> p r fo", fo=8),
           gL[0:8, OFF_C3:OFF_C3 + N_C3].rearrange("r (p fo) -> p r fo", fo=8))
        w.update(skip_k=skip_k, c1_k=c1_k, c2_w=c2_w, c3_w=c3_w)

    def surf(l, w, Ucs, sin_t):
        hw = HWS[l]
        wb_in = ins[f"wb{l}"]
        g0 = 0
        while g0 < hw:
            gn = min(1024, hw - g0)
            nsub = (gn + 511) // 512
            pss = [mk(psp, [F, 512], "wg", bufs=2) for _ in range(nsub)]
            for kc in range(8):
                wbt = mk(wbp, [128, 1024], "wbt", bf16)
                nc.sync.dma_start(wbt[:, :gn],
                                  wb_in[kc * 128:(kc + 1) * 128, g0:g0 + gn])
                for u in range(nsub):
                    un = min(512, gn - u * 512)
                    mm(pss[u][:, :un], w["btk"][:, kc * 64:(kc + 1) * 64],
                       wbt[:, u * 512:u * 512 + un], kc == 0, kc == 7)
            for u in range(nsub):
                un = min(512, gn - u * 512)
                lo = g0 + u * 512
                tt = mk(ab, [F, 512], "tt")
                nc.scalar.activation(tt[:, :un], pss[u][:, :un], ACTF.Sin,
                                     bias=w["btb_cos"][:])
                nc.vector.tensor_scalar(Ucs[0:64, lo:lo + un], tt[:, :un],
                                        w["cos_b"][:], None, ALU.subtract)
                tt2 = mk(ab, [F, 512], "tt")
                nc.scalar.activation(tt2[:, :un], pss[u][:, :un], ACTF.Sin,
                                     bias=w["b8"][0:64, 2:3])
                nc.gpsimd.tensor_scalar(sin_t[:, lo:lo + un], tt2[:, :un],
                                        w["sin_b"][:], None, ALU.subtract)
            g0 += gn
            pump(1)

    # per-layer state carried between calls
    trans_acc = [None]

    def call(l, c, w, Ucs, sin_t, so_prev):
        nonlocal out_t
        s = SIZES[l]
        hw = HWS[l]
        cn0 = min(512, hw)
        nch = max(1, hw // 512)
        out3 = out_t[:].rearrange("p (h w) -> p h w", h=s)

        # ---- sobel: t_hd = H-diff(out), t_h1 = H-121(out), u1/u2 vertical
        t_hd = mk(big, [F, hw], "thd", bf16)
        f3t = mk(big, [F, hw], "fscratch")
        t_h1 = mk(big, [F, hw], "to3", bf16)
        d3 = t_hd[:].rearrange("p (h w) -> p h w", h=s)
        f3 = f3t[:].rearrange("p (h w) -> p h w", h=s)
        o3 = t_h1[:].rearrange("p (h w) -> p h w", h=s)
        nc.vector.tensor_sub(d3[:, :, 1:s - 1], out3[:, :, 2:s],
                             out3[:, :, 0:s - 2])
        nc.gpsimd.tensor_copy(d3[:, :, 0:1], out3[:, :, 1:2])
        nc.gpsimd.tensor_scalar(d3[:, :, s - 1:s], out3[:, :, s - 2:s - 1],
                                -1.0, None, ALU.mult)
        nc.gpsimd.tensor_add(f3[:, :, 1:s - 1], out3[:, :, 2:s],
                             out3[:, :, 0:s - 2])
        nc.gpsimd.tensor_copy(f3[:, :, 0:1], out3[:, :, 1:2])
        nc.gpsimd.tensor_copy(f3[:, :, s - 1:s], out3[:, :, s - 2:s - 1])
        nc.vector.scalar_tensor_tensor(o3[:, :, :], out3[:, :, :], 2.0,
                                       f3[:, :, :], ALU.mult, ALU.add)
        u1 = mk(big, [F, hw], "u1", bf16)
        u2 = mk(big, [F, hw], "u2", bf16)
        u13 = u1[:].rearrange("p (h w) -> p h w", h=s)
        u23 = u2[:].rearrange("p (h w) -> p h w", h=s)
        nc.gpsimd.tensor_add(u13[:, 1:s - 1, :], d3[:, 0:s - 2, :],
                             d3[:, 2:s, :])
        nc.vector.tensor_copy(u13[:, 0:1, :], d3[:, 1:2, :])
        nc.vector.tensor_copy(u13[:, s - 1:s, :], d3[:, s - 2:s - 1, :])
        nc.vector.scalar_tensor_tensor(u13[:, :, :], d3[:, :, :], 2.0,
                                       u13[:, :, :], ALU.mult, ALU.add)
        nc.vector.tensor_sub(u23[:, 1:s - 1, :], o3[:, 2:s, :],
                             o3[:, 0:s - 2, :])
        nc.gpsimd.tensor_copy(u23[:, 0:1, :], o3[:, 1:2, :])
        nc.gpsimd.tensor_scalar(u23[:, s - 1:s, :], o3[:, s - 2:s - 1, :],
                                -1.0, None, ALU.mult)

        # ---- wave coefficient: so = sum(out) over space
        if so_prev is None:
            so = mk(smc, [F, 1], "so")
            nc.vector.tensor_reduce(so[:], out3, axis=AX.XY, op=ALU.add)
        else:
            so = so_prev
        so_bf = mk(smc, [F, 1], "so_bf", bf16)
        nc.gpsimd.tensor_scalar(so_bf[:], so[:], 1.0, None, ALU.mult)
        ps_c = mk(psp, [CPE, 1], "mini", bufs=1)
        mm(ps_c[:], w["c2w_w"][:], so_bf[:], True, True)
        coefv = mk(smc, [CPE, 1], "coefv")
        nc.scalar.activation(coefv[:], ps_c[:], ACTF.Identity,
                             bias=w["b8"][0:32, 3:4])
        coef96 = mk(smc, [96, 1], "coef96")
        nc.gpsimd.dma_start(coef96[64:96, :], coefv[:])
        nc.vector.tensor_scalar(w["weT"][64:96, :], w["w2ca"][64:96, :],
                                coef96[64:96, 0:1], None, ALU.mult)

        # ---- we/ge matmuls: drains on scalar, bn_stats on vector
        we_raw = mk(big, [128, hw], "weraw", bf16)
        ge_raw = mk(big, [128, hw], "geraw", bf16)
        we_st = mk(smc, [128, 6 * nch], "west")
        ge_st = mk(smc, [128, 6 * nch], "gest")
        for n in range(nch):
            c0 = n * cn0
            cn = cn0
            ps_we = mk(psp, [128, 512], "wg", bufs=2)
            mm(ps_we[:, :cn], w["weT"][:], Ucs[:, c0:c0 + cn], True, False)
            mm(ps_we[:, :cn], w["w2cbs"][:], sin_t[:, c0:c0 + cn], False, True)
            nc.vector.bn_stats(we_st[:, n * 6:(n + 1) * 6], ps_we[:, :cn])
            nc.scalar.activation(we_raw[:, c0:c0 + cn], ps_we[:, :cn],
                                 ACTF.Identity)
        for n in range(nch):
            c0 = n * cn0
            cn = cn0
            ps_ge = mk(psp, [128, 512], "wg", bufs=2)
            mm(ps_ge[:, :cn], w["wA"][:], u1[:, c0:c0 + cn], True, False)
            mm(ps_ge[:, :cn], w["wB"][:], u2[:, c0:c0 + cn], False, True)
            nc.vector.bn_stats(ge_st[:, n * 6:(n + 1) * 6], ps_ge[:, :cn])
            nc.scalar.activation(ge_raw[:, c0:c0 + cn], ps_ge[:, :cn],
                                 ACTF.Identity)

        # ---- instance-norm stats -> fold into skip/c1 weights
        def stats(st, nm):
            mv = mk(smc, [128, 2], f"mv{nm}")
            nc.vector.bn_aggr(mv[:], st[:])
            ve = mk(smc, [128, 1], f"ve{nm}")
            nc.vector.tensor_scalar(ve[:], mv[:, 1:2], 1e-5, None, ALU.add)
            nc.vector.reciprocal(ve[:], ve[:])
            rs = mk(smc, [128, 1], f"rs{nm}")
            nc.scalar.sqrt(rs[:], ve[:])
            nb = mk(smc, [128, 1], f"nb{nm}", bf16)
            nc.vector.scalar_tensor_tensor(nb[:], mv[:, 0:1], -1.0, rs[:],
                                           ALU.mult, ALU.mult)
            return rs, nb

        rs_ge, nb_ge = stats(ge_st, "g")
        rs_we, nb_we = stats(we_st, "w")

        skip_s = mk(gw, [128, 128], "skips", bf16)
        nc.vector.tensor_scalar(skip_s[:, 0:64], w["skip_k"][:, 0:64],
                                rs_ge[:], None, ALU.mult)
        nc.gpsimd.tensor_scalar(skip_s[:, 64:128], w["skip_k"][:, 64:128],
                                rs_we[:], None, ALU.mult)
        c1_s = mk(gw, [128, 256], "c1s", bf16)
        nc.vector.tensor_scalar(c1_s[:, 0:128], w["c1_k"][:, 0:128],
                                rs_ge[:], None, ALU.mult)
        nc.gpsimd.tensor_scalar(c1_s[:, 128:256], w["c1_k"][:, 128:256],
                                rs_we[:], None, ALU.mult)
        psF = mk(psp, [F, 1], "mini", bufs=1)
        mm(psF[:], w["skip_k"][:, 0:64], nb_ge[:], True, False)
        mm(psF[:], w["skip_k"][:, 64:128], nb_we[:], False, True)
        bsc = mk(smc, [F, 1], "bsc")
        nc.scalar.activation(bsc[:], psF[:], ACTF.Identity, bias=w["skipc3"][:])
        nc.vector.tensor_mul(bsc[:], bsc[:], leak64[:])
        psF2 = mk(psp, [128, 1], "mini", bufs=1)
        mm(psF2[:], w["c1_k"][:, 0:128], nb_ge[:], True, False)
        mm(psF2[:], w["c1_k"][:, 128:256], nb_we[:], False, True)
        c1b = mk(smc, [128, 1], "c1b")
        nc.scalar.activation(c1b[:], psF2[:], ACTF.Identity,
                             bias=w["b8"][:, 0:1])

        # ---- residual: skip + c1/lrelu/c2/lrelu/c3, update out
        last_img = (l == L - 1 and c == 1)
        tnsum = mk(smc, [F, 8], "tnsum") if c == 0 else None
        for n in range(nch):
            c0 = n * cn0
            cn = cn0
            ps_s = mk(psp, [F, 512], "sc", bufs=2)
            mm(ps_s[:, :cn], skip_s[:, 0:64], ge_raw[:, c0:c0 + cn], True,
               False)
            mm(ps_s[:, :cn], skip_s[:, 64:128], we_raw[:, c0:c0 + cn], False,
               False)
            ps_1 = mk(psp, [128, 512], "pa", bufs=3)
            mm(ps_1[:, :cn], c1_s[:, 0:128], ge_raw[:, c0:c0 + cn], True,
               False)
            mm(ps_1[:, :cn], c1_s[:, 128:256], we_raw[:, c0:c0 + cn], False,
               True)
            a1 = mk(ab, [128, 512], "a1", bf16)
            nc.scalar.activation(a1[:, :cn], ps_1[:, :cn], ACTF.Identity,
                                 bias=c1b[:])
            nc.vector.scalar_tensor_tensor(a1[:, :cn], a1[:, :cn], 0.2,
                                           a1[:, :cn], ALU.mult, ALU.max)
            ps_2 = mk(psp, [128, 512], "wg", bufs=2)
            mm(ps_2[:, :cn], w["c2_w"][:], a1[:, :cn], True, True)
            a2 = mk(ab, [128, 512], "a2", bf16)
            nc.scalar.activation(a2[:, :cn], ps_2[:, :cn], ACTF.Identity,
                                 bias=w["b8"][:, 1:2])
            nc.vector.scalar_tensor_tensor(a2[:, :cn], a2[:, :cn], 0.2,
                                           a2[:, :cn], ALU.mult, ALU.max)
            mm(ps_s[:, :cn], w["c3_w"][:], a2[:, :cn], False, True)
            tn = mk(ab, [F, 512], "tn")
            if tnsum is not None:
                nc.scalar.activation(tn[:, :cn], ps_s[:, :cn], ACTF.Identity,
                                     bias=bsc[:], scale=leak64[:],
                                     accum_out=tnsum[:, n:n + 1])
            else:
                nc.scalar.activation(tn[:, :cn], ps_s[:, :cn], ACTF.Identity,
                                     bias=bsc[:], scale=leak64[:])
            nc.vector.tensor_add(out_t[:, c0:c0 + cn],
                                 out_t[:, c0:c0 + cn], tn[:, :cn])
            if last_img:
                ob = mk(ab, [F, 512], "ob", bf16)
                nc.scalar.copy(ob[:, :cn], out_t[:, c0:c0 + cn])
                ps_i = mk(psp, [3, 512], "mini", bufs=1)
                mm(ps_i[:, :cn], imgWT_bf[:], ob[:, :cn], True, True)
                imgc = mk(ab, [3, 512], "imgc")
                nc.scalar.activation(imgc[:, :cn], ps_i[:, :cn], ACTF.Identity,
                                     bias=imgb[:])
                nc.vector.tensor_scalar(imgc[:, :cn], imgc[:, :cn], -1.0, 1.0,
                                        ALU.max, ALU.min)
                nc.sync.dma_start(out_img[:, c0:c0 + cn], imgc[:, :cn])
        if tnsum is None:
            return None
        red = mk(smc, [F, 1], "sored")
        nc.vector.tensor_reduce(red[:], tnsum[:, 0:nch], axis=AX.X, op=ALU.add)
        so_next = mk(smc, [F, 1], "sonext")
        nc.vector.tensor_add(so_next[:], so[:], red[:])
        return so_next

    def transition(l):
        nonlocal out_t
        s = SIZES[l]
        s2 = 2 * s
        x = out_t[:].rearrange("p (h w) -> p h w", h=s)
        upv = mk(big, [F, s2 * s], "u1")
        v = upv[:].rearrange("p (h w) -> p h w", h=s2)
        # vertical polyphase (values scaled by 1/UC2)
        nc.vector.scalar_tensor_tensor(v[:, 2:s2 - 2:2, :], x[:, 0:s - 2, :],
                                       UA, x[:, 2:s, :], ALU.mult, ALU.add)
        nc.vector.scalar_tensor_tensor(v[:, 2:s2 - 2:2, :], x[:, 1:s - 1, :],
                                       UBC, v[:, 2:s2 - 2:2, :], ALU.mult,
                                       ALU.add)
        nc.vector.scalar_tensor_tensor(v[:, 3:s2 - 2:2, :], x[:, 2:s, :], UA,
                                       x[:, 0:s - 2, :], ALU.mult, ALU.add)
        nc.vector.scalar_tensor_tensor(v[:, 3:s2 - 2:2, :], x[:, 1:s - 1, :],
                                       UBC, v[:, 3:s2 - 2:2, :], ALU.mult,
                                       ALU.add)
        nc.vector.scalar_tensor_tensor(v[:, 0:1, :], x[:, 0:1, :], UE00,
                                       x[:, 1:2, :], ALU.mult, ALU.add)
        nc.gpsimd.tensor_scalar(v[:, 1:2, :], x[:, 1:2, :], UA, None,
                                ALU.mult)
        nc.vector.scalar_tensor_tensor(v[:, 1:2, :], x[:, 0:1, :], UE1C,
                                       v[:, 1:2, :], ALU.mult, ALU.add)
        nc.gpsimd.tensor_scalar(v[:, s2 - 2:s2 - 1, :], x[:, s - 1:s, :],
                                UE1C, None, ALU.mult)
        nc.vector.scalar_tensor_tensor(v[:, s2 - 2:s2 - 1, :],
                                       x[:, s - 2:s - 1, :], UA,
                                       v[:, s2 - 2:s2 - 1, :], ALU.mult,
                                       ALU.add)
        nc.vector.scalar_tensor_tensor(v[:, s2 - 1:s2, :], x[:, s - 1:s, :],
                                       UE2C, x[:, s - 2:s - 1, :], ALU.mult,
                                       ALU.add)
        # horizontal polyphase
        uph = mk(big, [F, s2 * s2], "fscratch")
        h3 = uph[:].rearrange("p (h w) -> p h w", h=s2)
        nc.vector.scalar_tensor_tensor(h3[:, :, 2:s2 - 2:2], v[:, :, 0:s - 2],
                                       UA, v[:, :, 2:s], ALU.mult, ALU.add)
        nc.vector.scalar_tensor_tensor(h3[:, :, 2:s2 - 2:2], v[:, :, 1:s - 1],
                                       UBC, h3[:, :, 2:s2 - 2:2], ALU.mult,
                                       ALU.add)
        nc.vector.scalar_tensor_tensor(h3[:, :, 3:s2 - 2:2], v[:, :, 2:s], UA,
                                       v[:, :, 0:s - 2], ALU.mult, ALU.add)
        nc.vector.scalar_tensor_tensor(h3[:, :, 3:s2 - 2:2], v[:, :, 1:s - 1],
                                       UBC, h3[:, :, 3:s2 - 2:2], ALU.mult,
                                       ALU.add)
        nc.vector.scalar_tensor_tensor(h3[:, :, 0:1], v[:, :, 0:1], UE00,
                                       v[:, :, 1:2], ALU.mult, ALU.add)
        nc.gpsimd.tensor_scalar(h3[:, :, 1:2], v[:, :, 1:2], UA, None,
                                ALU.mult)
        nc.vector.scalar_tensor_tensor(h3[:, :, 1:2], v[:, :, 0:1], UE1C,
                                       h3[:, :, 1:2], ALU.mult, ALU.add)
        nc.gpsimd.tensor_scalar(h3[:, :, s2 - 2:s2 - 1], v[:, :, s - 1:s],
                                UE1C, None, ALU.mult)
        nc.vector.scalar_tensor_tensor(h3[:, :, s2 - 2:s2 - 1],
                                       v[:, :, s - 2:s - 1], UA,
                                       h3[:, :, s2 - 2:s2 - 1], ALU.mult,
                                       ALU.add)
        nc.vector.scalar_tensor_tensor(h3[:, :, s2 - 1:s2], v[:, :, s - 1:s],
                                       UE2C, v[:, :, s - 2:s - 1], ALU.mult,
                                       ALU.add)
        newout = mk(big, [F, s2 * s2], "out_b" if l == 0 else "out_a")
        tacc = mk(smc, [F, 1], "tacc")
        nc.scalar.activation(newout[:], uph[:], ACTF.Copy,
                             scale=UC2 * UC2, accum_out=tacc[:])
        trans_acc[0] = tacc
        out_t = newout

    # ---------------- main schedule
    # Pre-emit phase A through layer-2-early so the PE fills the initial
    # barrier shadow; the rest (l2 late) pumps inside phase B of layer 0.
    pump(7 + 1 + 5 + 1)          # E0 + a2aE0 + L0 + a2aL0
    pump(7 + 1 + 5 + 1)          # E1 + a2aE1 + L1 + a2aL1
    pump(7 + 1)                  # E2 + a2aE2
    for l in range(L):
        hw = HWS[l]
        w = load_early(l)
        load_late(l, w)
        Ucs = mk(big, [96, hw], "ucs", bf16)
        nc.sync.dma_start(Ucs[64:96, :], ins[f"cw{l}"][:, :])
        sin_t = mk(big, [F, hw], "sin", bf16)
        surf(l, w, Ucs, sin_t)
        pump(2)
        so0 = call(l, 0, w, Ucs, sin_t,
                   trans_acc[0] if l > 0 else None)
        pump(2)
        call(l, 1, w, Ucs, sin_t, so0)
        pump(2)
        if l < L - 1:
            transition(l)

    for p in reversed(ctxs):
        p.__exit__(None, None, None)


_MODULE_CACHE = {}


def build_module():
    if "nc" in _MODULE_CACHE:
        return _MODULE_CACHE["nc"]
    from concourse import bacc, mybir
    import concourse.tile as tile

    nc = bacc.Bacc("TRN2", target_bir_lowering=False, debug=False,
                   num_devices=NCORE)
    f32 = mybir.dt.float32
    dts = {"f32": mybir.dt.float32, "bf16": mybir.dt.bfloat16}
    ins = {}
    for name, shape, dt in INPUT_SHAPES:
        ins[name] = nc.dram_tensor(name, list(shape), dts[dt],
                                   kind="ExternalInput").ap()
    out_img = nc.dram_tensor("img", [3, HWS[-1]], f32,
                             kind="ExternalOutput").ap()
    with tile.TileContext(nc) as tc:
        build_kernel(tc, ins, out_img)
    nc.compile()
    _MODULE_CACHE["nc"] = nc
    return nc


# ---------------------------------------------------------------- host prep
def host_prep(inputs):
    """Full (unsharded) numpy inputs -> per-core in_maps."""
    lat = np.asarray(inputs["lat"], np.float32)
    leak = float(np.clip(np.asarray(inputs["leak_factor"]), 0.001, 1000.0))
    seed = np.asarray(inputs["seed"], np.float32)[0].reshape(F, 256)
    frac_W = np.asarray(inputs["frac_W"], np.float32)
    frac_b = np.asarray(inputs["frac_b"], np.float32)

    def T(W, fin, fout):
        W = np.asarray(W, np.float32)
        return W.reshape(L, LAT, fout, fin).transpose(0, 1, 3, 2)

    bt_T = T(inputs["bt_W"], 1024, F)
    c2w_T = T(inputs["c2w_W"], F, CPE).copy()
    for l in range(L):
        c2w_T[l] *= 1.0 / HWS[l]
    w2c_T = T(inputs["w2c_W"], CPE + 2 * F, 2 * F)
    w2cA_T = np.ascontiguousarray(w2c_T[:, :, :CPE, :])
    w2cB_T = np.ascontiguousarray(w2c_T[:, :, CPE:, :]) * np.float32(E1)
    g2c_T = T(inputs["g2c_W"], 2 * F, 2 * F)
    skip_T = T(inputs["cc_skip_W"], 4 * F, F)
    c1_T = T(inputs["cc_c1_W"], 4 * F, 2 * F)
    c2_T = T(inputs["cc_c2_W"], 2 * F, 2 * F)
    c3_T = T(inputs["cc_c3_W"], 2 * F, F)

    # bias columns: (L, LAT, 8, 128); device reads b8[p, r] = col r value p
    bias_cols = np.zeros((L, LAT, 8, 128), np.float32)
    bias_cols[:, :, 0, :] = np.asarray(inputs["cc_c1_b"], np.float32)
    bias_cols[:, :, 1, :] = np.asarray(inputs["cc_c2_b"], np.float32)
    bias_cols[:, :, 2, 0:64] = np.asarray(inputs["bt_b"], np.float32)
    bias_cols[:, :, 3, 0:32] = np.asarray(inputs["c2w_b"], np.float32)
    bias_cols[:, :, 4, 0:64] = np.asarray(inputs["cc_skip_b"], np.float32)
    bias_cols[:, :, 5, 0:64] = np.asarray(inputs["cc_c3_b"], np.float32)

    def fout_shard(Tm, j, fo):
        blk = Tm[:, :, :, j * fo:(j + 1) * fo]  # (L, LAT, fin, fo)
        return np.ascontiguousarray(blk).reshape(L, LAT, -1)

    import ml_dtypes

    bf16 = ml_dtypes.bfloat16
    common = {
        "latT": np.ascontiguousarray(lat.T),
        "fracW": frac_W,
        "fracb": frac_b,
        "leak": np.full((1, 1), leak, np.float32),
        "seed": np.ascontiguousarray(seed),
        "imgWT": np.ascontiguousarray(np.asarray(inputs["img_W"],
                                                 np.float32).T),
        "imgb": np.asarray(inputs["img_b"], np.float32).reshape(3, 1),
    }
    for l in range(L):
        common[f"wb{l}"] = np.ascontiguousarray(
            np.asarray(inputs[f"wave_bias{l}"], np.float32)[0].reshape(
                F * 16, HWS[l])).astype(bf16)
        common[f"cw{l}"] = pos_enc_np(SIZES[l]).astype(bf16)

    in_maps = []
    for j in range(NCORE):
        wwsE = np.concatenate([
            np.ascontiguousarray(
                bt_T[:, :, j * 128:(j + 1) * 128, :]).reshape(L, LAT, N_BT),
            fout_shard(c2w_T, j, 4),
            fout_shard(w2cA_T, j, 16),
            fout_shard(w2cB_T, j, 16),
            fout_shard(g2c_T, j, 16),
            np.ascontiguousarray(bias_cols[:, :, j, :]),
        ], axis=2).astype(bf16)
        assert wwsE.shape == (L, LAT, SCOLS_E), wwsE.shape
        wwsL = np.concatenate([
            fout_shard(skip_T, j, 8),
            fout_shard(c1_T, j, 16),
            fout_shard(c2_T, j, 16),
            fout_shard(c3_T, j, 8),
        ], axis=2).astype(bf16)
        assert wwsL.shape == (L, LAT, SCOLS_L), wwsL.shape
        m = dict(common)
        m["wwsE"] = wwsE
        m["wwsL"] = wwsL
        in_maps.append(m)
    return in_maps


def kernel(**inputs):
    from concourse.bass_utils import run_bass_kernel_spmd

    nc = build_module()
    in_maps = host_prep(inputs)
    res = run_bass_kernel_spmd(nc, in_maps, list(range(NCORE)))
    imgs = [res.results[b]["img"].reshape(3, SIZES[-1], SIZES[-1])
            for b in range(B)]
    return np.stack(imgs).astype(np.float32)


if __name__ == "__main__":
    nc = build_module()
    print("module built ok; instructions:",
          sum(len(bb.instructions) for bb in nc.main_func.blocks))


# revision 19
# speedup vs baseline: 1.4284x; 1.1845x over previous
"""Trainium2 Bass kernel for nn_Decoder_72911364816952 (v2).

Strategy (8 NeuronCores, memory-bound regime):
  Phase A (all cores): stream the dynamic-weight generator matrices Ww
    (bf16, 1/8 column-shard per core, host-sliced) through the PE with the
    lat_new vectors stationary (M=8 = all destination ranks at once).
    Columns are split into an EARLY block (bt/c2w/w2c/g2c + biases) and a
    LATE block (skip/c1/c2/c3) with separate AllToAlls per layer so phase B
    can start as soon as the early weights arrive.
  Phase B (each core, one sample): surf/sin/cos wave path, Sobel folded
    into the g2c matmuls, instance norm FOLDED INTO the skip/c1 weights
    (scale lhsT rows by rs, bias fixup via N=1 matmuls), fused
    bilinear-2x-upsample+gauss as a 3-tap polyphase filter, elementwise
    work spread across Vector/GpSimd/Scalar engines, channel sums for the
    wave coefficient and instance norms picked up for free via accum_out.
"""
import math
import sys

sys.path.insert(0, "/opt/trn_rl_repo")

import numpy as np

# ---------------------------------------------------------------- constants
L = 3
LAT = 256
F = 64
CPE = 32
B = 4
NCORE = 8
SIZES = [16, 32, 64]
HWS = [s * s for s in SIZES]
K_FREQ = 8

# EARLY block per-core column layout (offsets in elements)
OFF_BT, N_BT = 0, 8192          # fin-shard: rank r = fin rows r*128.., (il=128, o=64)
OFF_C2W, N_C2W = 8192, 256      # fout-shard fo=4 (p=64, fo=4)
OFF_W2CA, N_W2CA = 8448, 512    # fout-shard fo=16 (p=32, fo=16)
OFF_W2CB, N_W2CB = 8960, 2048   # fout-shard fo=16, 2 chunks (c=2, p=64, fo=16)
OFF_G2C, N_G2C = 11008, 2048    # fout-shard fo=16, 2 chunks (c=2, p=64, fo=16)
OFF_BIAS, N_BIAS = 13056, 128   # bias column j from rank j
SCOLS_E = 13184
# LATE block
OFF_SKIP, N_SKIP = 0, 2048      # fo=8, 2 chunks (c=2, p=128, fo=8)
OFF_C1, N_C1 = 2048, 4096       # fo=16, 2 chunks (c=2, p=128, fo=16)
OFF_C2, N_C2 = 6144, 2048       # fo=16, 1 chunk
OFF_C3, N_C3 = 8192, 1024      # fo=8, 1 chunk
SCOLS_L = 9216
SIM_LRELU = True  # True: STT lrelu (CoreSim-compatible); False: fused scalar Lrelu

E1 = math.e
_ga_raw = math.exp(-0.5)
GA = _ga_raw / (1.0 + 2.0 * _ga_raw)   # normalized 1D gauss edge tap (a)
GB = 1.0 / (1.0 + 2.0 * _ga_raw)       # center tap (b)
# composite bilinear-2x + gauss 3-tap polyphase coefficients
UC0 = 0.75 * GA + 0.25 * GB
UC1 = GA + 0.75 * GB
UC2 = 0.25 * GA
UA = UC0 / UC2
UBC = UC1 / UC2
UE00 = (GB + 0.75 * GA) / UC2
UE1C = (1.25 * GA + 0.75 * GB) / UC2
UE2C = (0.75 * GA + GB) / UC2
PI_2 = math.pi / 2.0


def pos_enc_np(size):
    p = np.arange(size, dtype=np.float32)
    feats = []
    for k in range(K_FREQ):
        ang = p * (2.0 ** k) * (2.0 * np.pi / size)
        s, c = np.sin(ang).astype(np.float32), np.cos(ang).astype(np.float32)
        feats += [np.broadcast_to(s[:, None], (size, size)),
                  np.broadcast_to(c[:, None], (size, size)),
                  np.broadcast_to(s[None, :], (size, size)),
                  np.broadcast_to(c[None, :], (size, size))]
    return np.ascontiguousarray(np.stack(feats).reshape(4 * K_FREQ, size * size))


INPUT_SHAPES = [
    ("wwsE", (L, LAT, SCOLS_E), "bf16"),
    ("wwsL", (L, LAT, SCOLS_L), "bf16"),
    ("latT", (LAT, B), "f32"),
    ("fracW", (L, LAT, LAT), "f32"),
    ("fracb", (L, LAT), "f32"),
    ("leak", (1, 1), "f32"),
    ("seed", (F, 256), "f32"),
    ("wb0", (1024, HWS[0]), "bf16"),
    ("wb1", (1024, HWS[1]), "bf16"),
    ("wb2", (1024, HWS[2]), "bf16"),
    ("cw0", (CPE, HWS[0]), "bf16"),
    ("cw1", (CPE, HWS[1]), "bf16"),
    ("cw2", (CPE, HWS[2]), "bf16"),
    ("imgWT", (F, 3), "f32"),
    ("imgb", (3, 1), "f32"),
]


# ---------------------------------------------------------------- device code
def build_kernel(tc, ins, out_img):
    import concourse.bass as bass
    from concourse import mybir

    nc = tc.nc
    f32 = mybir.dt.float32
    bf16 = mybir.dt.bfloat16
    ALU = mybir.AluOpType
    ACTF = mybir.ActivationFunctionType
    AX = mybir.AxisListType
    ctxs = []

    def pool(name, bufs, space="SBUF"):
        p = tc.tile_pool(name=name, bufs=bufs, space=space)
        ctxs.append(p)
        return p.__enter__()

    def mk(pool_, shape, tag, dt=None, bufs=None):
        return pool_.tile(shape, dt or f32, name=tag, tag=tag, bufs=bufs)

    dram = pool("dram", 1, "DRAM")
    psp = pool("psp", 1, "PSUM")
    big = pool("big", 1)
    gw = pool("gw", 2)
    ab = pool("ab", 3)
    sm = pool("sm", 1)
    smc = pool("smc", 2)
    pha = pool("pha", 2)
    stg = pool("stg", 2)
    wbp = pool("wbp", 3)

    def mm(ps, lhsT, rhs, start, stop):
        nc.tensor.matmul(ps, lhsT, rhs, start=start, stop=stop)

    # ---------------- setup: latT, leak, lat_new8 per layer
    latT = []
    for kc in range(2):
        t = mk(sm, [128, B], f"latT{kc}")
        nc.sync.dma_start(t[:], ins["latT"][kc * 128:(kc + 1) * 128, :])
        latT.append(t)

    leak_sb = mk(sm, [1, 1], "leak")
    nc.sync.dma_start(leak_sb[:], ins["leak"][:, :])
    ones64 = mk(sm, [1, F], "ones64")
    nc.vector.memset(ones64[:], 1.0)
    ps_l = mk(psp, [F, 1], "mini", bufs=1)
    mm(ps_l[:], ones64[:], leak_sb[:], True, True)
    leak64 = mk(sm, [F, 1], "leak64")
    nc.scalar.copy(leak64[:], ps_l[:])

    lat8 = []  # [l][kc] -> (128, 8) bf16 stationary tiles
    for l in range(L):
        fb = []
        for kc in range(2):
            t = mk(sm, [128, 1], f"fracb{l}{kc}")
            nc.sync.dma_start(
                t[:], ins["fracb"][l, kc * 128:(kc + 1) * 128].rearrange(
                    "(p u) -> p u", u=1))
            fb.append(t)
        lnT = []
        for mc in range(2):
            ps = mk(psp, [128, B], "mini", bufs=1)
            for kc in range(2):
                fw = mk(sm, [128, 128], "fwtile", bufs=2)
                nc.sync.dma_start(
                    fw[:], ins["fracW"][l, kc * 128:(kc + 1) * 128,
                                        mc * 128:(mc + 1) * 128])
                mm(ps[:], fw[:], latT[kc][:], kc == 0, kc == 1)
            t = mk(sm, [128, 8], f"latnew{l}{mc}", bf16)
            nc.scalar.activation(t[:, 0:4], ps[:], ACTF.Identity, bias=fb[mc][:])
            nc.scalar.activation(t[:, 4:8], ps[:], ACTF.Identity, bias=fb[mc][:])
            lnT.append(t)
        lat8.append(lnT)

    imgWT = mk(sm, [F, 3], "imgWT")
    nc.sync.dma_start(imgWT[:], ins["imgWT"][:, :])
    imgWT_bf = mk(sm, [F, 3], "imgWT_bf", bf16)
    nc.vector.tensor_copy(imgWT_bf[:], imgWT[:])
    imgb = mk(sm, [3, 1], "imgb")
    nc.sync.dma_start(imgb[:], ins["imgb"][:, :])

    # out state: starts as seed
    out_t = mk(big, [F, 256], "out_a")
    nc.sync.dma_start(out_t[:], ins["seed"][:, :])

    # ---------------- phase A task machinery
    gatE = {}
    gatL = {}
    _di = [0]

    def pa_chunk(wws, l, contrib, c0, dc):
        rhs0 = mk(pha, [128, 2048], "rhs0", bf16)
        rhs1 = mk(pha, [128, 2048], "rhs1", bf16)
        nc.sync.dma_start(rhs0[:, :dc], wws[l, 0:128, c0:c0 + dc])
        nc.sync.dma_start(rhs1[:, :dc], wws[l, 128:256, c0:c0 + dc])
        stage = mk(stg, [8, 2048], "stage", bf16)
        n0 = 0
        while n0 < dc:
            nn = min(512, dc - n0)
            ps = mk(psp, [8, 512], "pa", bufs=3)
            mm(ps[:, :nn], lat8[l][0][:], rhs0[:, n0:n0 + nn], True, False)
            mm(ps[:, :nn], lat8[l][1][:], rhs1[:, n0:n0 + nn], False, True)
            if _di[0] % 2 == 0:
                nc.scalar.copy(stage[:, n0:n0 + nn], ps[:, :nn])
            else:
                nc.vector.tensor_copy(stage[:, n0:n0 + nn], ps[:, :nn])
            _di[0] += 1
            n0 += nn
        nc.scalar.dma_start(contrib[0:8, c0:c0 + dc], stage[:, :dc])

    def pa_a2a(contrib, gathered):
        nc.gpsimd.collective_compute(
            "AllToAll", mybir.AluOpType.bypass,
            replica_groups=[list(range(NCORE))],
            ins=[contrib.opt()],
            outs=[gathered.opt()],
        )

    pa_tasks = []

    def build_layer_tasks(l, block):
        wws = ins["wwsE"] if block == "E" else ins["wwsL"]
        scols = SCOLS_E if block == "E" else SCOLS_L
        contrib = mk(dram, [NCORE, scols], f"contrib{block}{l}", bf16)
        gathered = mk(dram, [NCORE, scols], f"a2a{block}{l}", bf16)
        (gatE if block == "E" else gatL)[l] = gathered
        c0 = 0
        while c0 < scols:
            dc = min(2048, scols - c0)
            pa_tasks.append(
                lambda wws=wws, l=l, contrib=contrib, c0=c0, dc=dc:
                pa_chunk(wws, l, contrib, c0, dc))
            c0 += dc
        pa_tasks.append(lambda c=contrib, g=gathered: pa_a2a(c, g))

    for l in range(L):
        build_layer_tasks(l, "E")
        build_layer_tasks(l, "L")

    def pump(n):
        for _ in range(n):
            if pa_tasks:
                pa_tasks.pop(0)()

    # ---------------- phase B helpers
    def ld(dst, src):
        nc.gpsimd.dma_start(dst, src)

    def load_early(l):
        gE = gatE[l]
        w = {}
        btk = mk(gw, [128, 512], "btk", bf16)
        ld(btk[:].rearrange("p (r o) -> p r o", o=64),
           gE[0:8, OFF_BT:OFF_BT + N_BT].rearrange("r (il o) -> il r o", o=64))
        c2w_w = mk(gw, [64, 32], "c2ww", bf16)
        ld(c2w_w[:].rearrange("p (r fo) -> p r fo", fo=4),
           gE[0:8, OFF_C2W:OFF_C2W + N_C2W].rearrange("r (p fo) -> p r fo", fo=4))
        w2ca = mk(gw, [96, 128], "w2ca", bf16)
        ld(w2ca[64:96, :].rearrange("p (r fo) -> p r fo", fo=16),
           gE[0:8, OFF_W2CA:OFF_W2CA + N_W2CA].rearrange(
               "r (p fo) -> p r fo", fo=16))
        weT = mk(gw, [96, 128], "weT", bf16)
        ld(weT[0:64, :].rearrange("p (r fo) -> p r fo", fo=16),
           gE[0:8, OFF_W2CB:OFF_W2CB + 1024].rearrange(
               "r (p fo) -> p r fo", fo=16))
        w2cbs = mk(gw, [64, 128], "w2cbs", bf16)
        ld(w2cbs[:].rearrange("p (r fo) -> p r fo", fo=16),
           gE[0:8, OFF_W2CB + 1024:OFF_W2CB + 2048].rearrange(
               "r (p fo) -> p r fo", fo=16))
        g2ca = mk(gw, [64, 128], "g2ca", bf16)
        ld(g2ca[:].rearrange("p (r fo) -> p r fo", fo=16),
           gE[0:8, OFF_G2C:OFF_G2C + 1024].rearrange(
               "r (p fo) -> p r fo", fo=16))
        g2cb = mk(gw, [64, 128], "g2cb", bf16)
        ld(g2cb[:].rearrange("p (r fo) -> p r fo", fo=16),
           gE[0:8, OFF_G2C + 1024:OFF_G2C + 2048].rearrange(
               "r (p fo) -> p r fo", fo=16))
        b8h = mk(gw, [128, 8], "b8h", bf16)
        ld(b8h[:], gE[0:8, OFF_BIAS:OFF_BIAS + N_BIAS].rearrange("r p -> p r"))
        b8 = mk(gw, [128, 8], "b8")
        nc.vector.tensor_copy(b8[:], b8h[:])
        wA = mk(gw, [64, 128], "wA", bf16)
        nc.vector.tensor_scalar(wA[:], g2ca[:], 0.125, None, ALU.mult)
        wB = mk(gw, [64, 128], "wB", bf16)
        nc.vector.tensor_scalar(wB[:], g2cb[:], 0.125, None, ALU.mult)
        btb_cos = mk(gw, [64, 1], "btbc")
        nc.vector.tensor_scalar(btb_cos[:], b8[0:64, 2:3], PI_2, None, ALU.add)
        cos_b = mk(gw, [64, 1], "cos_b")
        nc.scalar.activation(cos_b[:], btb_cos[:], ACTF.Sin)
        sin_b = mk(gw, [64, 1], "sin_b")
        nc.scalar.activation(sin_b[:], b8[0:64, 2:3], ACTF.Sin)
        skipc3 = mk(gw, [64, 1], "skc3")
        nc.vector.tensor_add(skipc3[:], b8[0:64, 4:5], b8[0:64, 5:6])
        w.update(btk=btk, c2w_w=c2w_w, w2ca=w2ca, weT=weT, w2cbs=w2cbs,
                 wA=wA, wB=wB, b8=b8, btb_cos=btb_cos, cos_b=cos_b,
                 sin_b=sin_b, skipc3=skipc3)
        return w

    def load_late(l, w):
        gL = gatL[l]
        skip_k = mk(gw, [128, 128], "skipk", bf16)
        for c in range(2):
            ld(skip_k[:, c * 64:(c + 1) * 64].rearrange(
                   "p (r fo) -> p r fo", fo=8),
               gL[0:8, OFF_SKIP + c * 1024:OFF_SKIP + (c + 1) * 1024].rearrange(
                   "r (p fo) -> p r fo", fo=8))
        c1_k = mk(gw, [128, 256], "c1k", bf16)
        for c in range(2):
            ld(c1_k[:, c * 128:(c + 1) * 128].rearrange(
                   "p (r fo) -> p r fo", fo=16),
               gL[0:8, OFF_C1 + c * 2048:OFF_C1 + (c + 1) * 2048].rearrange(
                   "r (p fo) -> p r fo", fo=16))
        c2_w = mk(gw, [128, 128], "c2w_", bf16)
        ld(c2_w[:].rearrange("p (r fo) -> p r fo", fo=16),
           gL[0:8, OFF_C2:OFF_C2 + N_C2].rearrange("r (p fo) -> p r fo", fo=16))
        c3_w = mk(gw, [128, 64], "c3w", bf16)
        ld(c3_w[:].rearrange("p (r fo) -> p r fo", fo=8),
           gL[0:8, OFF_C3:OFF_C3 + N_C3].rearrange("r (p fo) -> p r fo", fo=8))
        w.update(skip_k=skip_k, c1_k=c1_k, c2_w=c2_w, c3_w=c3_w)

    def surf(l, w, Ucs, sin_t):
        hw = HWS[l]
        wb_in = ins[f"wb{l}"]
        g0 = 0
        while g0 < hw:
            gn = min(1024, hw - g0)
            nsub = (gn + 511) // 512
            pss = [mk(psp, [F, 512], "wg", bufs=2) for _ in range(nsub)]
            for kc in range(8):
                wbt = mk(wbp, [128, 1024], "wbt", bf16)
                nc.sync.dma_start(wbt[:, :gn],
                                  wb_in[kc * 128:(kc + 1) * 128, g0:g0 + gn])
                for u in range(nsub):
                    un = min(512, gn - u * 512)
                    mm(pss[u][:, :un], w["btk"][:, kc * 64:(kc + 1) * 64],
                       wbt[:, u * 512:u * 512 + un], kc == 0, kc == 7)
            for u in range(nsub):
                un = min(512, gn - u * 512)
                lo = g0 + u * 512
                tt = mk(ab, [F, 512], "tt")
                nc.scalar.activation(tt[:, :un], pss[u][:, :un], ACTF.Sin,
                                     bias=w["btb_cos"][:])
                nc.vector.tensor_scalar(Ucs[0:64, lo:lo + un], tt[:, :un],
                                        w["cos_b"][:], None, ALU.subtract)
                tt2 = mk(ab, [F, 512], "tt")
                nc.scalar.activation(tt2[:, :un], pss[u][:, :un], ACTF.Sin,
                                     bias=w["b8"][0:64, 2:3])
                nc.vector.tensor_scalar(sin_t[:, lo:lo + un], tt2[:, :un],
                                        w["sin_b"][:], None, ALU.subtract)
            g0 += gn
            pump(1)

    # per-layer state carried between calls
    trans_acc = [None]

    def call(l, c, w, Ucs, sin_t, so_prev, ob_in, ob_out):
        nonlocal out_t
        s = SIZES[l]
        hw = HWS[l]
        cn0 = min(512, hw)
        nch = max(1, hw // 512)
        out3 = out_t[:].rearrange("p (h w) -> p h w", h=s)

        # ---- sobel (pure bf16 on DVE; edge rows/cols via scalar engine)
        ob3 = ob_in[:].rearrange("p (h w) -> p h w", h=s)
        t_hd = mk(big, [F, hw], "thd", bf16)
        f3t = mk(big, [F, hw], "tf3b", bf16)
        t_h1 = mk(big, [F, hw], "to3", bf16)
        d3 = t_hd[:].rearrange("p (h w) -> p h w", h=s)
        f3 = f3t[:].rearrange("p (h w) -> p h w", h=s)
        o3 = t_h1[:].rearrange("p (h w) -> p h w", h=s)
        nc.vector.tensor_sub(d3[:, :, 1:s - 1], ob3[:, :, 2:s],
                             ob3[:, :, 0:s - 2])
        nc.scalar.activation(d3[:, :, 0:1], ob3[:, :, 1:2], ACTF.Identity)
        nc.scalar.activation(d3[:, :, s - 1:s], ob3[:, :, s - 2:s - 1],
                             ACTF.Identity, scale=-1.0)
        nc.vector.tensor_add(f3[:, :, 1:s - 1], ob3[:, :, 2:s],
                             ob3[:, :, 0:s - 2])
        nc.scalar.activation(f3[:, :, 0:1], ob3[:, :, 1:2], ACTF.Identity)
        nc.scalar.activation(f3[:, :, s - 1:s], ob3[:, :, s - 2:s - 1],
                             ACTF.Identity)
        nc.vector.scalar_tensor_tensor(o3[:, :, :], ob3[:, :, :], 2.0,
                                       f3[:, :, :], ALU.mult, ALU.add)
        u1 = mk(big, [F, hw], "u1", bf16)
        u2 = mk(big, [F, hw], "u2", bf16)
        u13 = u1[:].rearrange("p (h w) -> p h w", h=s)
        u23 = u2[:].rearrange("p (h w) -> p h w", h=s)
        nc.vector.tensor_add(u13[:, 1:s - 1, :], d3[:, 0:s - 2, :],
                             d3[:, 2:s, :])
        nc.scalar.activation(u13[:, 0:1, :], d3[:, 1:2, :], ACTF.Identity)
        nc.scalar.activation(u13[:, s - 1:s, :], d3[:, s - 2:s - 1, :],
                             ACTF.Identity)
        nc.vector.scalar_tensor_tensor(u13[:, :, :], d3[:, :, :], 2.0,
                                       u13[:, :, :], ALU.mult, ALU.add)
        nc.vector.tensor_sub(u23[:, 1:s - 1, :], o3[:, 2:s, :],
                             o3[:, 0:s - 2, :])
        nc.scalar.activation(u23[:, 0:1, :], o3[:, 1:2, :], ACTF.Identity)
        nc.scalar.activation(u23[:, s - 1:s, :], o3[:, s - 2:s - 1, :],
                             ACTF.Identity, scale=-1.0)

        # ---- wave coefficient: so = sum(out) over space
        if so_prev is None:
            so = mk(smc, [F, 1], "so")
            nc.vector.tensor_reduce(so[:], out3, axis=AX.XY, op=ALU.add)
        else:
            so = so_prev
        so_bf = mk(smc, [F, 1], "so_bf", bf16)
        nc.vector.tensor_copy(so_bf[:], so[:])
        ps_c = mk(psp, [CPE, 1], "mini", bufs=1)
        mm(ps_c[:], w["c2w_w"][:], so_bf[:], True, True)
        coefv = mk(smc, [CPE, 1], "coefv")
        nc.scalar.activation(coefv[:], ps_c[:], ACTF.Identity,
                             bias=w["b8"][0:32, 3:4])
        coef96 = mk(smc, [96, 1], "coef96")
        nc.gpsimd.dma_start(coef96[64:96, :], coefv[:])
        nc.vector.tensor_scalar(w["weT"][64:96, :], w["w2ca"][64:96, :],
                                coef96[64:96, 0:1], None, ALU.mult)

        # ---- we/ge matmuls: drains on scalar, bn_stats on vector
        we_raw = mk(big, [128, hw], "weraw", bf16)
        ge_raw = mk(big, [128, hw], "geraw", bf16)
        we_st = mk(smc, [128, 6 * nch], "west")
        ge_st = mk(smc, [128, 6 * nch], "gest")
        for n in range(nch):
            c0 = n * cn0
            cn = cn0
            ps_we = mk(psp, [128, 512], "wg", bufs=2)
            mm(ps_we[:, :cn], w["weT"][:], Ucs[:, c0:c0 + cn], True, False)
            mm(ps_we[:, :cn], w["w2cbs"][:], sin_t[:, c0:c0 + cn], False, True)
            nc.scalar.activation(we_raw[:, c0:c0 + cn], ps_we[:, :cn],
                                 ACTF.Identity)
            nc.vector.bn_stats(we_st[:, n * 6:(n + 1) * 6],
                               we_raw[:, c0:c0 + cn])
        for n in range(nch):
            c0 = n * cn0
            cn = cn0
            ps_ge = mk(psp, [128, 512], "wg", bufs=2)
            mm(ps_ge[:, :cn], w["wA"][:], u1[:, c0:c0 + cn], True, False)
            mm(ps_ge[:, :cn], w["wB"][:], u2[:, c0:c0 + cn], False, True)
            nc.scalar.activation(ge_raw[:, c0:c0 + cn], ps_ge[:, :cn],
                                 ACTF.Identity)
            nc.vector.bn_stats(ge_st[:, n * 6:(n + 1) * 6],
                               ge_raw[:, c0:c0 + cn])

        # ---- instance-norm stats -> fold into skip/c1 weights
        def stats(st, nm):
            mv = mk(smc, [128, 2], f"mv{nm}")
            nc.vector.bn_aggr(mv[:], st[:])
            ve = mk(smc, [128, 1], f"ve{nm}")
            nc.vector.tensor_scalar(ve[:], mv[:, 1:2], 1e-5, None, ALU.add)
            nc.vector.reciprocal(ve[:], ve[:])
            rs = mk(smc, [128, 1], f"rs{nm}")
            nc.scalar.sqrt(rs[:], ve[:])
            nb = mk(smc, [128, 1], f"nb{nm}", bf16)
            nc.vector.scalar_tensor_tensor(nb[:], mv[:, 0:1], -1.0, rs[:],
                                           ALU.mult, ALU.mult)
            return rs, nb

        rs_ge, nb_ge = stats(ge_st, "g")
        rs_we, nb_we = stats(we_st, "w")

        skip_s = mk(gw, [128, 128], "skips", bf16)
        nc.vector.tensor_scalar(skip_s[:, 0:64], w["skip_k"][:, 0:64],
                                rs_ge[:], None, ALU.mult)
        nc.vector.tensor_scalar(skip_s[:, 64:128], w["skip_k"][:, 64:128],
                                rs_we[:], None, ALU.mult)
        c1_s = mk(gw, [128, 256], "c1s", bf16)
        nc.vector.tensor_scalar(c1_s[:, 0:128], w["c1_k"][:, 0:128],
                                rs_ge[:], None, ALU.mult)
        nc.vector.tensor_scalar(c1_s[:, 128:256], w["c1_k"][:, 128:256],
                                rs_we[:], None, ALU.mult)
        psF = mk(psp, [F, 1], "mini", bufs=1)
        mm(psF[:], w["skip_k"][:, 0:64], nb_ge[:], True, False)
        mm(psF[:], w["skip_k"][:, 64:128], nb_we[:], False, True)
        bsc = mk(smc, [F, 1], "bsc")
        nc.scalar.activation(bsc[:], psF[:], ACTF.Identity, bias=w["skipc3"][:])
        nc.vector.tensor_mul(bsc[:], bsc[:], leak64[:])
        psF2 = mk(psp, [128, 1], "mini", bufs=1)
        mm(psF2[:], w["c1_k"][:, 0:128], nb_ge[:], True, False)
        mm(psF2[:], w["c1_k"][:, 128:256], nb_we[:], False, True)
        c1b = mk(smc, [128, 1], "c1b")
        nc.scalar.activation(c1b[:], psF2[:], ACTF.Identity,
                             bias=w["b8"][:, 0:1])

        # ---- residual: skip + c1/lrelu/c2/lrelu/c3, update out
        last_img = (l == L - 1 and c == 1)
        tnsum = mk(smc, [F, 8], "tnsum") if c == 0 else None
        for n in range(nch):
            c0 = n * cn0
            cn = cn0
            ps_s = mk(psp, [F, 512], "sc", bufs=2)
            mm(ps_s[:, :cn], skip_s[:, 0:64], ge_raw[:, c0:c0 + cn], True,
               False)
            mm(ps_s[:, :cn], skip_s[:, 64:128], we_raw[:, c0:c0 + cn], False,
               False)
            ps_1 = mk(psp, [128, 512], "pa", bufs=3)
            mm(ps_1[:, :cn], c1_s[:, 0:128], ge_raw[:, c0:c0 + cn], True,
               False)
            mm(ps_1[:, :cn], c1_s[:, 128:256], we_raw[:, c0:c0 + cn], False,
               True)
            a1 = mk(ab, [128, 512], "a1", bf16)
            if SIM_LRELU:
                nc.scalar.activation(a1[:, :cn], ps_1[:, :cn], ACTF.Identity,
                                     bias=c1b[:])
                nc.vector.scalar_tensor_tensor(a1[:, :cn], a1[:, :cn], 0.2,
                                               a1[:, :cn], ALU.mult, ALU.max)
            else:
                nc.scalar.activation(a1[:, :cn], ps_1[:, :cn], ACTF.Lrelu,
                                     bias=c1b[:], alpha=0.2)
            ps_2 = mk(psp, [128, 512], "wg", bufs=2)
            mm(ps_2[:, :cn], w["c2_w"][:], a1[:, :cn], True, True)
            a2 = mk(ab, [128, 512], "a2", bf16)
            if SIM_LRELU:
                nc.scalar.activation(a2[:, :cn], ps_2[:, :cn], ACTF.Identity,
                                     bias=w["b8"][:, 1:2])
                nc.vector.scalar_tensor_tensor(a2[:, :cn], a2[:, :cn], 0.2,
                                               a2[:, :cn], ALU.mult, ALU.max)
            else:
                nc.scalar.activation(a2[:, :cn], ps_2[:, :cn], ACTF.Lrelu,
                                     bias=w["b8"][:, 1:2], alpha=0.2)
            mm(ps_s[:, :cn], w["c3_w"][:], a2[:, :cn], False, True)
            tn = mk(ab, [F, 512], "tn")
            if tnsum is not None:
                nc.scalar.activation(tn[:, :cn], ps_s[:, :cn], ACTF.Identity,
                                     bias=bsc[:], scale=leak64[:],
                                     accum_out=tnsum[:, n:n + 1])
            else:
                nc.scalar.activation(tn[:, :cn], ps_s[:, :cn], ACTF.Identity,
                                     bias=bsc[:], scale=leak64[:])
            nc.vector.tensor_add(out_t[:, c0:c0 + cn],
                                 out_t[:, c0:c0 + cn], tn[:, :cn])
            if ob_out is not None:
                nc.scalar.copy(ob_out[:, c0:c0 + cn], out_t[:, c0:c0 + cn])
            if last_img:
                ob = mk(ab, [F, 512], "ob", bf16)
                nc.scalar.copy(ob[:, :cn], out_t[:, c0:c0 + cn])
                ps_i = mk(psp, [3, 512], "mini", bufs=1)
                mm(ps_i[:, :cn], imgWT_bf[:], ob[:, :cn], True, True)
                imgc = mk(ab, [3, 512], "imgc")
                nc.scalar.activation(imgc[:, :cn], ps_i[:, :cn], ACTF.Identity,
                                     bias=imgb[:])
                nc.vector.tensor_scalar(imgc[:, :cn], imgc[:, :cn], -1.0, 1.0,
                                        ALU.max, ALU.min)
                nc.sync.dma_start(out_img[:, c0:c0 + cn], imgc[:, :cn])
        if tnsum is None:
            return None
        red = mk(smc, [F, 1], "sored")
        nc.vector.tensor_reduce(red[:], tnsum[:, 0:nch], axis=AX.X, op=ALU.add)
        so_next = mk(smc, [F, 1], "sonext")
        nc.vector.tensor_add(so_next[:], so[:], red[:])
        return so_next

    def transition(l):
        nonlocal out_t
        s = SIZES[l]
        s2 = 2 * s
        x = out_t[:].rearrange("p (h w) -> p h w", h=s)
        upv = mk(big, [F, s2 * s], "u1")
        v = upv[:].rearrange("p (h w) -> p h w", h=s2)
        # vertical polyphase (values scaled by 1/UC2)
        nc.vector.scalar_tensor_tensor(v[:, 2:s2 - 2:2, :], x[:, 0:s - 2, :],
                                       UA, x[:, 2:s, :], ALU.mult, ALU.add)
        nc.vector.scalar_tensor_tensor(v[:, 2:s2 - 2:2, :], x[:, 1:s - 1, :],
                                       UBC, v[:, 2:s2 - 2:2, :], ALU.mult,
                                       ALU.add)
        nc.vector.scalar_tensor_tensor(v[:, 3:s2 - 2:2, :], x[:, 2:s, :], UA,
                                       x[:, 0:s - 2, :], ALU.mult, ALU.add)
        nc.vector.scalar_tensor_tensor(v[:, 3:s2 - 2:2, :], x[:, 1:s - 1, :],
                                       UBC, v[:, 3:s2 - 2:2, :], ALU.mult,
                                       ALU.add)
        nc.vector.scalar_tensor_tensor(v[:, 0:1, :], x[:, 0:1, :], UE00,
                                       x[:, 1:2, :], ALU.mult, ALU.add)
        nc.vector.tensor_scalar(v[:, 1:2, :], x[:, 1:2, :], UA, None,
                                ALU.mult)
        nc.vector.scalar_tensor_tensor(v[:, 1:2, :], x[:, 0:1, :], UE1C,
                                       v[:, 1:2, :], ALU.mult, ALU.add)
        nc.vector.tensor_scalar(v[:, s2 - 2:s2 - 1, :], x[:, s - 1:s, :],
                                UE1C, None, ALU.mult)
        nc.vector.scalar_tensor_tensor(v[:, s2 - 2:s2 - 1, :],
                                       x[:, s - 2:s - 1, :], UA,
                                       v[:, s2 - 2:s2 - 1, :], ALU.mult,
                                       ALU.add)
        nc.vector.scalar_tensor_tensor(v[:, s2 - 1:s2, :], x[:, s - 1:s, :],
                                       UE2C, x[:, s - 2:s - 1, :], ALU.mult,
                                       ALU.add)
        # horizontal polyphase
        uph = mk(big, [F, s2 * s2], "fscratch")
        h3 = uph[:].rearrange("p (h w) -> p h w", h=s2)
        nc.vector.scalar_tensor_tensor(h3[:, :, 2:s2 - 2:2], v[:, :, 0:s - 2],
                                       UA, v[:, :, 2:s], ALU.mult, ALU.add)
        nc.vector.scalar_tensor_tensor(h3[:, :, 2:s2 - 2:2], v[:, :, 1:s - 1],
                                       UBC, h3[:, :, 2:s2 - 2:2], ALU.mult,
                                       ALU.add)
        nc.vector.scalar_tensor_tensor(h3[:, :, 3:s2 - 2:2], v[:, :, 2:s], UA,
                                       v[:, :, 0:s - 2], ALU.mult, ALU.add)
        nc.vector.scalar_tensor_tensor(h3[:, :, 3:s2 - 2:2], v[:, :, 1:s - 1],
                                       UBC, h3[:, :, 3:s2 - 2:2], ALU.mult,
                                       ALU.add)
        nc.vector.scalar_tensor_tensor(h3[:, :, 0:1], v[:, :, 0:1], UE00,
                                       v[:, :, 1:2], ALU.mult, ALU.add)
        nc.vector.tensor_scalar(h3[:, :, 1:2], v[:, :, 1:2], UA, None,
                                ALU.mult)
        nc.vector.scalar_tensor_tensor(h3[:, :, 1:2], v[:, :, 0:1], UE1C,
                                       h3[:, :, 1:2], ALU.mult, ALU.add)
        nc.vector.tensor_scalar(h3[:, :, s2 - 2:s2 - 1], v[:, :, s - 1:s],
                                UE1C, None, ALU.mult)
        nc.vector.scalar_tensor_tensor(h3[:, :, s2 - 2:s2 - 1],
                                       v[:, :, s - 2:s - 1], UA,
                                       h3[:, :, s2 - 2:s2 - 1], ALU.mult,
                                       ALU.add)
        nc.vector.scalar_tensor_tensor(h3[:, :, s2 - 1:s2], v[:, :, s - 1:s],
                                       UE2C, v[:, :, s - 2:s - 1], ALU.mult,
                                       ALU.add)
        newout = mk(big, [F, s2 * s2], "out_b" if l == 0 else "out_a")
        tacc = mk(smc, [F, 1], "tacc")
        nc.scalar.activation(newout[:], uph[:], ACTF.Copy,
                             scale=UC2 * UC2, accum_out=tacc[:])
        trans_acc[0] = tacc
        out_t = newout

    # ---------------- main schedule
    # Pre-emit phase A through layer-2-early so the PE fills the initial
    # barrier shadow; the rest (l2 late) pumps inside phase B of layer 0.
    pump(7 + 1 + 5 + 1)          # E0 + a2aE0 + L0 + a2aL0
    pump(7 + 1 + 5 + 1)          # E1 + a2aE1 + L1 + a2aL1
    pump(7 + 1)                  # E2 + a2aE2
    for l in range(L):
        hw = HWS[l]
        w = load_early(l)
        load_late(l, w)
        Ucs = mk(big, [96, hw], "ucs", bf16)
        nc.sync.dma_start(Ucs[64:96, :], ins[f"cw{l}"][:, :])
        sin_t = mk(big, [F, hw], "sin", bf16)
        surf(l, w, Ucs, sin_t)
        obf0 = mk(big, [F, hw], "obfa", bf16)
        nc.scalar.copy(obf0[:], out_t[:])
        obf1 = mk(big, [F, hw], "obfb", bf16)
        pump(2)
        so0 = call(l, 0, w, Ucs, sin_t,
                   trans_acc[0] if l > 0 else None, obf0, obf1)
        pump(2)
        call(l, 1, w, Ucs, sin_t, so0, obf1, None)
        pump(2)
        if l < L - 1:
            transition(l)

    for p in reversed(ctxs):
        p.__exit__(None, None, None)


_MODULE_CACHE = {}


def build_module():
    if "nc" in _MODULE_CACHE:
        return _MODULE_CACHE["nc"]
    from concourse import bacc, mybir
    import concourse.tile as tile

    nc = bacc.Bacc("TRN2", target_bir_lowering=False, debug=False,
                   num_devices=NCORE)
    f32 = mybir.dt.float32
    dts = {"f32": mybir.dt.float32, "bf16": mybir.dt.bfloat16}
    ins = {}
    for name, shape, dt in INPUT_SHAPES:
        ins[name] = nc.dram_tensor(name, list(shape), dts[dt],
                                   kind="ExternalInput").ap()
    out_img = nc.dram_tensor("img", [3, HWS[-1]], f32,
                             kind="ExternalOutput").ap()
    with tile.TileContext(nc) as tc:
        build_kernel(tc, ins, out_img)
    nc.compile()
    _MODULE_CACHE["nc"] = nc
    return nc


# ---------------------------------------------------------------- host prep
def host_prep(inputs):
    """Full (unsharded) numpy inputs -> per-core in_maps."""
    lat = np.asarray(inputs["lat"], np.float32)
    leak = float(np.clip(np.asarray(inputs["leak_factor"]), 0.001, 1000.0))
    seed = np.asarray(inputs["seed"], np.float32)[0].reshape(F, 256)
    frac_W = np.asarray(inputs["frac_W"], np.float32)
    frac_b = np.asarray(inputs["frac_b"], np.float32)

    def T(W, fin, fout):
        W = np.asarray(W, np.float32)
        return W.reshape(L, LAT, fout, fin).transpose(0, 1, 3, 2)

    bt_T = T(inputs["bt_W"], 1024, F)
    c2w_T = T(inputs["c2w_W"], F, CPE).copy()
    for l in range(L):
        c2w_T[l] *= 1.0 / HWS[l]
    w2c_T = T(inputs["w2c_W"], CPE + 2 * F, 2 * F)
    w2cA_T = np.ascontiguousarray(w2c_T[:, :, :CPE, :])
    w2cB_T = np.ascontiguousarray(w2c_T[:, :, CPE:, :]) * np.float32(E1)
    g2c_T = T(inputs["g2c_W"], 2 * F, 2 * F)
    skip_T = T(inputs["cc_skip_W"], 4 * F, F)
    c1_T = T(inputs["cc_c1_W"], 4 * F, 2 * F)
    c2_T = T(inputs["cc_c2_W"], 2 * F, 2 * F)
    c3_T = T(inputs["cc_c3_W"], 2 * F, F)

    # bias columns: (L, LAT, 8, 128); device reads b8[p, r] = col r value p
    bias_cols = np.zeros((L, LAT, 8, 128), np.float32)
    bias_cols[:, :, 0, :] = np.asarray(inputs["cc_c1_b"], np.float32)
    bias_cols[:, :, 1, :] = np.asarray(inputs["cc_c2_b"], np.float32)
    bias_cols[:, :, 2, 0:64] = np.asarray(inputs["bt_b"], np.float32)
    bias_cols[:, :, 3, 0:32] = np.asarray(inputs["c2w_b"], np.float32)
    bias_cols[:, :, 4, 0:64] = np.asarray(inputs["cc_skip_b"], np.float32)
    bias_cols[:, :, 5, 0:64] = np.asarray(inputs["cc_c3_b"], np.float32)

    def fout_shard(Tm, j, fo):
        blk = Tm[:, :, :, j * fo:(j + 1) * fo]  # (L, LAT, fin, fo)
        return np.ascontiguousarray(blk).reshape(L, LAT, -1)

    import ml_dtypes

    bf16 = ml_dtypes.bfloat16
    common = {
        "latT": np.ascontiguousarray(lat.T),
        "fracW": frac_W,
        "fracb": frac_b,
        "leak": np.full((1, 1), leak, np.float32),
        "seed": np.ascontiguousarray(seed),
        "imgWT": np.ascontiguousarray(np.asarray(inputs["img_W"],
                                                 np.float32).T),
        "imgb": np.asarray(inputs["img_b"], np.float32).reshape(3, 1),
    }
    for l in range(L):
        common[f"wb{l}"] = np.ascontiguousarray(
            np.asarray(inputs[f"wave_bias{l}"], np.float32)[0].reshape(
                F * 16, HWS[l])).astype(bf16)
        common[f"cw{l}"] = pos_enc_np(SIZES[l]).astype(bf16)

    in_maps = []
    for j in range(NCORE):
        wwsE = np.concatenate([
            np.ascontiguousarray(
                bt_T[:, :, j * 128:(j + 1) * 128, :]).reshape(L, LAT, N_BT),
            fout_shard(c2w_T, j, 4),
            fout_shard(w2cA_T, j, 16),
            fout_shard(w2cB_T, j, 16),
            fout_shard(g2c_T, j, 16),
            np.ascontiguousarray(bias_cols[:, :, j, :]),
        ], axis=2).astype(bf16)
        assert wwsE.shape == (L, LAT, SCOLS_E), wwsE.shape
        wwsL = np.concatenate([
            fout_shard(skip_T, j, 8),
            fout_shard(c1_T, j, 16),
            fout_shard(c2_T, j, 16),
            fout_shard(c3_T, j, 8),
        ], axis=2).astype(bf16)
        assert wwsL.shape == (L, LAT, SCOLS_L), wwsL.shape
        m = dict(common)
        m["wwsE"] = wwsE
        m["wwsL"] = wwsL
        in_maps.append(m)
    return in_maps


def kernel(**inputs):
    from concourse.bass_utils import run_bass_kernel_spmd

    nc = build_module()
    in_maps = host_prep(inputs)
    res = run_bass_kernel_spmd(nc, in_maps, list(range(NCORE)))
    imgs = [res.results[b]["img"].reshape(3, SIZES[-1], SIZES[-1])
            for b in range(B)]
    return np.stack(imgs).astype(np.float32)


if __name__ == "__main__":
    nc = build_module()
    print("module built ok; instructions:",
          sum(len(bb.instructions) for bb in nc.main_func.blocks))


# revision 25
# speedup vs baseline: 1.4817x; 1.0373x over previous
"""Trainium2 Bass kernel for nn_Decoder_72911364816952 (v2).

Strategy (8 NeuronCores, memory-bound regime):
  Phase A (all cores): stream the dynamic-weight generator matrices Ww
    (bf16, 1/8 column-shard per core, host-sliced) through the PE with the
    lat_new vectors stationary (M=8 = all destination ranks at once).
    Columns are split into an EARLY block (bt/c2w/w2c/g2c + biases) and a
    LATE block (skip/c1/c2/c3) with separate AllToAlls per layer so phase B
    can start as soon as the early weights arrive.
  Phase B (each core, one sample): surf/sin/cos wave path, Sobel folded
    into the g2c matmuls, instance norm FOLDED INTO the skip/c1 weights
    (scale lhsT rows by rs, bias fixup via N=1 matmuls), fused
    bilinear-2x-upsample+gauss as a 3-tap polyphase filter, elementwise
    work spread across Vector/GpSimd/Scalar engines, channel sums for the
    wave coefficient and instance norms picked up for free via accum_out.
"""
import math
import sys

sys.path.insert(0, "/opt/trn_rl_repo")

import numpy as np

# ---------------------------------------------------------------- constants
L = 3
LAT = 256
F = 64
CPE = 32
B = 4
NCORE = 8
SIZES = [16, 32, 64]
HWS = [s * s for s in SIZES]
K_FREQ = 8

# EARLY block per-core column layout (offsets in elements)
OFF_BT, N_BT = 0, 8192          # fin-shard: rank r = fin rows r*128.., (il=128, o=64)
OFF_C2W, N_C2W = 8192, 256      # fout-shard fo=4 (p=64, fo=4)
OFF_W2CA, N_W2CA = 8448, 512    # fout-shard fo=16 (p=32, fo=16)
OFF_W2CB, N_W2CB = 8960, 2048   # fout-shard fo=16, 2 chunks (c=2, p=64, fo=16)
OFF_G2C, N_G2C = 11008, 2048    # fout-shard fo=16, 2 chunks (c=2, p=64, fo=16)
OFF_BIAS, N_BIAS = 13056, 128   # bias column j from rank j
SCOLS_E = 13184
# LATE block
OFF_SKIP, N_SKIP = 0, 2048      # fo=8, 2 chunks (c=2, p=128, fo=8)
OFF_C1, N_C1 = 2048, 4096       # fo=16, 2 chunks (c=2, p=128, fo=16)
OFF_C2, N_C2 = 6144, 2048       # fo=16, 1 chunk
OFF_C3, N_C3 = 8192, 1024      # fo=8, 1 chunk
SCOLS_L = 9216
SIM_LRELU = True  # True: STT lrelu (CoreSim-compatible); False: fused scalar Lrelu
# fp8 phase-A per-block power-of-2 scales (host multiplies Ww, device
# divides at the psum-drain activations)
S_BT = 128.0
S_C2W = (1.0, 1.0, 1.0)
S_W2C = 1.0
S_G2C = 1.0
S_SC = 1.0
S_C1 = 1.0
S_C2 = 1.0
S_B = 1.0
N_RESTE = SCOLS_E - N_BT  # 4992

E1 = math.e
_ga_raw = math.exp(-0.5)
GA = _ga_raw / (1.0 + 2.0 * _ga_raw)   # normalized 1D gauss edge tap (a)
GB = 1.0 / (1.0 + 2.0 * _ga_raw)       # center tap (b)
# composite bilinear-2x + gauss 3-tap polyphase coefficients
UC0 = 0.75 * GA + 0.25 * GB
UC1 = GA + 0.75 * GB
UC2 = 0.25 * GA
UA = UC0 / UC2
UBC = UC1 / UC2
UE00 = (GB + 0.75 * GA) / UC2
UE1C = (1.25 * GA + 0.75 * GB) / UC2
UE2C = (0.75 * GA + GB) / UC2
PI_2 = math.pi / 2.0


def pos_enc_np(size):
    p = np.arange(size, dtype=np.float32)
    feats = []
    for k in range(K_FREQ):
        ang = p * (2.0 ** k) * (2.0 * np.pi / size)
        s, c = np.sin(ang).astype(np.float32), np.cos(ang).astype(np.float32)
        feats += [np.broadcast_to(s[:, None], (size, size)),
                  np.broadcast_to(c[:, None], (size, size)),
                  np.broadcast_to(s[None, :], (size, size)),
                  np.broadcast_to(c[None, :], (size, size))]
    return np.ascontiguousarray(np.stack(feats).reshape(4 * K_FREQ, size * size))


INPUT_SHAPES = [
    ("wwsB", (L, 128, 2 * N_BT), "fp8"),
    ("wwsE", (L, LAT, N_RESTE), "bf16"),
    ("wwsL", (L, LAT, SCOLS_L), "bf16"),
    ("latT", (LAT, B), "f32"),
    ("fracW", (L, LAT, LAT), "f32"),
    ("fracb", (L, LAT), "f32"),
    ("leak", (1, 1), "f32"),
    ("seed", (F, 256), "f32"),
    ("wb0", (1024, HWS[0]), "bf16"),
    ("wb1", (1024, HWS[1]), "bf16"),
    ("wb2", (1024, HWS[2]), "bf16"),
    ("cw0", (CPE, HWS[0]), "bf16"),
    ("cw1", (CPE, HWS[1]), "bf16"),
    ("cw2", (CPE, HWS[2]), "bf16"),
    ("imgWT", (F, 3), "f32"),
    ("imgb", (3, 1), "f32"),
]


# ---------------------------------------------------------------- device code
def build_kernel(tc, ins, out_img):
    import concourse.bass as bass
    from concourse import mybir

    nc = tc.nc
    f32 = mybir.dt.float32
    bf16 = mybir.dt.bfloat16
    ALU = mybir.AluOpType
    ACTF = mybir.ActivationFunctionType
    AX = mybir.AxisListType
    ctxs = []

    def pool(name, bufs, space="SBUF"):
        p = tc.tile_pool(name=name, bufs=bufs, space=space)
        ctxs.append(p)
        return p.__enter__()

    def mk(pool_, shape, tag, dt=None, bufs=None):
        return pool_.tile(shape, dt or f32, name=tag, tag=tag, bufs=bufs)

    dram = pool("dram", 1, "DRAM")
    psp = pool("psp", 1, "PSUM")
    big = pool("big", 1)
    gw = pool("gw", 2)
    ab = pool("ab", 3)
    sm = pool("sm", 1)
    smc = pool("smc", 2)
    pha = pool("pha", 2)
    stg = pool("stg", 2)
    wbp = pool("wbp", 3)

    def mm(ps, lhsT, rhs, start, stop):
        nc.tensor.matmul(ps, lhsT, rhs, start=start, stop=stop)

    # ---------------- setup: latT, leak, lat_new8 per layer
    latT = []
    for kc in range(2):
        t = mk(sm, [128, B], f"latT{kc}")
        nc.sync.dma_start(t[:], ins["latT"][kc * 128:(kc + 1) * 128, :])
        latT.append(t)

    leak_sb = mk(sm, [1, 1], "leak")
    nc.sync.dma_start(leak_sb[:], ins["leak"][:, :])
    ones64 = mk(sm, [1, F], "ones64")
    nc.vector.memset(ones64[:], 1.0)
    ps_l = mk(psp, [F, 1], "mini", bufs=1)
    mm(ps_l[:], ones64[:], leak_sb[:], True, True)
    leak64 = mk(sm, [F, 1], "leak64")
    nc.scalar.copy(leak64[:], ps_l[:])
    leak_sc = mk(sm, [F, 1], "leak_sc")
    nc.scalar.activation(leak_sc[:], ps_l[:], ACTF.Identity, scale=1.0 / S_SC)

    lat8 = []  # [l] -> (128, 16) fp8 DoubleRow stationary
    lat8bf = []  # [l][kc] -> (128, 8) bf16 stationary
    for l in range(L):
        fb = []
        for kc in range(2):
            t = mk(sm, [128, 1], f"fracb{l}{kc}")
            nc.sync.dma_start(
                t[:], ins["fracb"][l, kc * 128:(kc + 1) * 128].rearrange(
                    "(p u) -> p u", u=1))
            fb.append(t)
        t = mk(sm, [128, 64], f"latnew{l}", mybir.dt.float8e4)
        nc.vector.memset(t[:], 0.0)
        tb = []
        for mc in range(2):
            ps = mk(psp, [128, B], "mini", bufs=1)
            for kc in range(2):
                fw = mk(sm, [128, 128], "fwtile", bufs=2)
                nc.sync.dma_start(
                    fw[:], ins["fracW"][l, kc * 128:(kc + 1) * 128,
                                        mc * 128:(mc + 1) * 128])
                mm(ps[:], fw[:], latT[kc][:], kc == 0, kc == 1)
            o0 = mc * 32
            nc.scalar.activation(t[:, o0:o0 + 4], ps[:], ACTF.Identity,
                                 bias=fb[mc][:])
            nc.scalar.activation(t[:, o0 + 4:o0 + 8], ps[:], ACTF.Identity,
                                 bias=fb[mc][:])
            tbm = mk(sm, [128, 8], f"latnbf{l}{mc}", bf16)
            nc.scalar.activation(tbm[:, 0:4], ps[:], ACTF.Identity,
                                 bias=fb[mc][:])
            nc.scalar.activation(tbm[:, 4:8], ps[:], ACTF.Identity,
                                 bias=fb[mc][:])
            tb.append(tbm)
        lat8.append(t)
        lat8bf.append(tb)

    imgWT = mk(sm, [F, 3], "imgWT")
    nc.sync.dma_start(imgWT[:], ins["imgWT"][:, :])
    imgWT_bf = mk(sm, [F, 3], "imgWT_bf", bf16)
    nc.vector.tensor_copy(imgWT_bf[:], imgWT[:])
    imgb = mk(sm, [3, 1], "imgb")
    nc.sync.dma_start(imgb[:], ins["imgb"][:, :])

    # out state: starts as seed
    out_t = mk(big, [F, 256], "out_a")
    nc.sync.dma_start(out_t[:], ins["seed"][:, :])

    # ---------------- phase A task machinery
    gatE = {}
    gatL = {}
    _di = [0]

    def pa_drain(stage, ps, n0, nn):
        if _di[0] % 2 == 0:
            nc.scalar.copy(stage[:, n0:n0 + nn], ps[:, :nn])
        else:
            nc.vector.tensor_copy(stage[:, n0:n0 + nn], ps[:, :nn])
        _di[0] += 1

    def pa_chunk8(l, contrib, c0, dc):
        rhs = mk(pha, [128, 4096], "rhs8", mybir.dt.float8e4)
        nc.sync.dma_start(rhs[:, :2 * dc],
                          ins["wwsB"][l, :, 2 * c0:2 * (c0 + dc)])
        lhsT3 = lat8[l][:].rearrange("p (two m) -> p two m", two=2)
        stage = mk(stg, [8, 2048], "stage", bf16)
        n0 = 0
        while n0 < dc:
            nn = min(512, dc - n0)
            ps = mk(psp, [32, 512], "pa", bufs=3)
            rhs3 = rhs[:, 2 * n0:2 * n0 + 2 * nn].rearrange(
                "p (two n) -> p two n", two=2)
            nc.tensor.matmul(ps[:, :nn], lhsT3, rhs3, start=True, stop=True,
                             perf_mode=mybir.MatmulPerfMode.DoubleRow)
            pa_drain(stage, ps[0:8, :], n0, nn)
            n0 += nn
        nc.scalar.dma_start(contrib[0:8, c0:c0 + dc], stage[:, :dc])

    def pa_chunk(wws, l, contrib, cbase, c0, dc):
        rhs0 = mk(pha, [128, 2048], "rhs0", bf16)
        rhs1 = mk(pha, [128, 2048], "rhs1", bf16)
        nc.sync.dma_start(rhs0[:, :dc], wws[l, 0:128, c0:c0 + dc])
        nc.sync.dma_start(rhs1[:, :dc], wws[l, 128:256, c0:c0 + dc])
        stage = mk(stg, [8, 2048], "stage", bf16)
        n0 = 0
        while n0 < dc:
            nn = min(512, dc - n0)
            ps = mk(psp, [8, 512], "pa", bufs=3)
            mm(ps[:, :nn], lat8bf[l][0][:], rhs0[:, n0:n0 + nn], True, False)
            mm(ps[:, :nn], lat8bf[l][1][:], rhs1[:, n0:n0 + nn], False, True)
            pa_drain(stage, ps, n0, nn)
            n0 += nn
        nc.scalar.dma_start(contrib[0:8, cbase + c0:cbase + c0 + dc],
                            stage[:, :dc])

    def pa_a2a(contrib, gathered):
        nc.gpsimd.collective_compute(
            "AllToAll", mybir.AluOpType.bypass,
            replica_groups=[list(range(NCORE))],
            ins=[contrib.opt()],
            outs=[gathered.opt()],
        )

    pa_tasks = []

    def build_layer_tasks(l, block):
        scols = SCOLS_E if block == "E" else SCOLS_L
        contrib = mk(dram, [NCORE, scols], f"contrib{block}{l}", bf16)
        gathered = mk(dram, [NCORE, scols], f"a2a{block}{l}", bf16)
        (gatE if block == "E" else gatL)[l] = gathered
        if block == "E":
            for c0 in range(0, N_BT, 2048):
                pa_tasks.append(
                    lambda l=l, contrib=contrib, c0=c0:
                    pa_chunk8(l, contrib, c0, 2048))
            c0 = 0
            while c0 < N_RESTE:
                dc = min(2048, N_RESTE - c0)
                pa_tasks.append(
                    lambda l=l, contrib=contrib, c0=c0, dc=dc:
                    pa_chunk(ins["wwsE"], l, contrib, N_BT, c0, dc))
                c0 += dc
        else:
            c0 = 0
            while c0 < scols:
                dc = min(2048, scols - c0)
                pa_tasks.append(
                    lambda l=l, contrib=contrib, c0=c0, dc=dc:
                    pa_chunk(ins["wwsL"], l, contrib, 0, c0, dc))
                c0 += dc
        pa_tasks.append(lambda c=contrib, g=gathered: pa_a2a(c, g))

    for l in range(L):
        build_layer_tasks(l, "E")
        build_layer_tasks(l, "L")

    def pump(n):
        for _ in range(n):
            if pa_tasks:
                pa_tasks.pop(0)()

    # ---------------- phase B helpers
    def ld(dst, src):
        nc.gpsimd.dma_start(dst, src)

    def load_early(l):
        gE = gatE[l]
        w = {}
        btk = mk(gw, [128, 512], "btk", bf16)
        ld(btk[:].rearrange("p (r o) -> p r o", o=64),
           gE[0:8, OFF_BT:OFF_BT + N_BT].rearrange("r (il o) -> il r o", o=64))
        c2w_w = mk(gw, [64, 32], "c2ww", bf16)
        ld(c2w_w[:].rearrange("p (r fo) -> p r fo", fo=4),
           gE[0:8, OFF_C2W:OFF_C2W + N_C2W].rearrange("r (p fo) -> p r fo", fo=4))
        w2ca = mk(gw, [96, 128], "w2ca", bf16)
        ld(w2ca[64:96, :].rearrange("p (r fo) -> p r fo", fo=16),
           gE[0:8, OFF_W2CA:OFF_W2CA + N_W2CA].rearrange(
               "r (p fo) -> p r fo", fo=16))
        weT = mk(gw, [96, 128], "weT", bf16)
        ld(weT[0:64, :].rearrange("p (r fo) -> p r fo", fo=16),
           gE[0:8, OFF_W2CB:OFF_W2CB + 1024].rearrange(
               "r (p fo) -> p r fo", fo=16))
        w2cbs = mk(gw, [64, 128], "w2cbs", bf16)
        ld(w2cbs[:].rearrange("p (r fo) -> p r fo", fo=16),
           gE[0:8, OFF_W2CB + 1024:OFF_W2CB + 2048].rearrange(
               "r (p fo) -> p r fo", fo=16))
        g2ca = mk(gw, [64, 128], "g2ca", bf16)
        ld(g2ca[:].rearrange("p (r fo) -> p r fo", fo=16),
           gE[0:8, OFF_G2C:OFF_G2C + 1024].rearrange(
               "r (p fo) -> p r fo", fo=16))
        g2cb = mk(gw, [64, 128], "g2cb", bf16)
        ld(g2cb[:].rearrange("p (r fo) -> p r fo", fo=16),
           gE[0:8, OFF_G2C + 1024:OFF_G2C + 2048].rearrange(
               "r (p fo) -> p r fo", fo=16))
        b8h = mk(gw, [128, 8], "b8h", bf16)
        ld(b8h[:], gE[0:8, OFF_BIAS:OFF_BIAS + N_BIAS].rearrange("r p -> p r"))
        b8 = mk(gw, [128, 8], "b8")
        nc.vector.tensor_scalar(b8[:], b8h[:], 1.0 / S_B, None, ALU.mult)
        btb_cos = mk(gw, [64, 1], "btbc")
        nc.vector.tensor_scalar(btb_cos[:], b8[0:64, 2:3], PI_2, None, ALU.add)
        cos_b = mk(gw, [64, 1], "cos_b")
        nc.scalar.activation(cos_b[:], btb_cos[:], ACTF.Sin)
        sin_b = mk(gw, [64, 1], "sin_b")
        nc.scalar.activation(sin_b[:], b8[0:64, 2:3], ACTF.Sin)
        skipc3 = mk(gw, [64, 1], "skc3")
        nc.vector.tensor_add(skipc3[:], b8[0:64, 4:5], b8[0:64, 5:6])
        w.update(btk=btk, c2w_w=c2w_w, w2ca=w2ca, weT=weT, w2cbs=w2cbs,
                 wA=g2ca, wB=g2cb, b8=b8, btb_cos=btb_cos, cos_b=cos_b,
                 sin_b=sin_b, skipc3=skipc3)
        return w

    def load_late(l, w):
        gL = gatL[l]
        skip_k = mk(gw, [128, 128], "skipk", bf16)
        for c in range(2):
            ld(skip_k[:, c * 64:(c + 1) * 64].rearrange(
                   "p (r fo) -> p r fo", fo=8),
               gL[0:8, OFF_SKIP + c * 1024:OFF_SKIP + (c + 1) * 1024].rearrange(
                   "r (p fo) -> p r fo", fo=8))
        c1_k = mk(gw, [128, 256], "c1k", bf16)
        for c in range(2):
            ld(c1_k[:, c * 128:(c + 1) * 128].rearrange(
                   "p (r fo) -> p r fo", fo=16),
               gL[0:8, OFF_C1 + c * 2048:OFF_C1 + (c + 1) * 2048].rearrange(
                   "r (p fo) -> p r fo", fo=16))
        c2_w = mk(gw, [128, 128], "c2w_", bf16)
        ld(c2_w[:].rearrange("p (r fo) -> p r fo", fo=16),
           gL[0:8, OFF_C2:OFF_C2 + N_C2].rearrange("r (p fo) -> p r fo", fo=16))
        c3_w = mk(gw, [128, 64], "c3w", bf16)
        ld(c3_w[:].rearrange("p (r fo) -> p r fo", fo=8),
           gL[0:8, OFF_C3:OFF_C3 + N_C3].rearrange("r (p fo) -> p r fo", fo=8))
        w.update(skip_k=skip_k, c1_k=c1_k, c2_w=c2_w, c3_w=c3_w)

    def surf(l, w, Ucs, sin_t):
        hw = HWS[l]
        wb_in = ins[f"wb{l}"]
        g0 = 0
        while g0 < hw:
            gn = min(1024, hw - g0)
            nsub = (gn + 511) // 512
            pss = [mk(psp, [F, 512], "wg", bufs=2) for _ in range(nsub)]
            for kc in range(8):
                wbt = mk(wbp, [128, 1024], "wbt", bf16)
                nc.sync.dma_start(wbt[:, :gn],
                                  wb_in[kc * 128:(kc + 1) * 128, g0:g0 + gn])
                for u in range(nsub):
                    un = min(512, gn - u * 512)
                    mm(pss[u][:, :un], w["btk"][:, kc * 64:(kc + 1) * 64],
                       wbt[:, u * 512:u * 512 + un], kc == 0, kc == 7)
            for u in range(nsub):
                un = min(512, gn - u * 512)
                lo = g0 + u * 512
                tt = mk(ab, [F, 512], "tt")
                nc.scalar.activation(tt[:, :un], pss[u][:, :un], ACTF.Sin,
                                     bias=w["btb_cos"][:], scale=1.0 / S_BT)
                nc.vector.tensor_scalar(Ucs[0:64, lo:lo + un], tt[:, :un],
                                        w["cos_b"][:], None, ALU.subtract)
                tt2 = mk(ab, [F, 512], "tt")
                nc.scalar.activation(tt2[:, :un], pss[u][:, :un], ACTF.Sin,
                                     bias=w["b8"][0:64, 2:3],
                                     scale=1.0 / S_BT)
                nc.vector.tensor_scalar(sin_t[:, lo:lo + un], tt2[:, :un],
                                        w["sin_b"][:], None, ALU.subtract)
            g0 += gn
            pump(1)

    # per-layer state carried between calls
    trans_acc = [None]

    def call(l, c, w, Ucs, sin_t, so_prev, ob_in, ob_out):
        nonlocal out_t
        s = SIZES[l]
        hw = HWS[l]
        cn0 = min(512, hw)
        nch = max(1, hw // 512)
        out3 = out_t[:].rearrange("p (h w) -> p h w", h=s)

        # ---- sobel (pure bf16 on DVE; edge rows/cols via scalar engine)
        ob3 = ob_in[:].rearrange("p (h w) -> p h w", h=s)
        t_hd = mk(big, [F, hw], "thd", bf16)
        f3t = mk(big, [F, hw], "tf3b", bf16)
        t_h1 = mk(big, [F, hw], "to3", bf16)
        d3 = t_hd[:].rearrange("p (h w) -> p h w", h=s)
        f3 = f3t[:].rearrange("p (h w) -> p h w", h=s)
        o3 = t_h1[:].rearrange("p (h w) -> p h w", h=s)
        nc.vector.tensor_sub(d3[:, :, 1:s - 1], ob3[:, :, 2:s],
                             ob3[:, :, 0:s - 2])
        nc.scalar.activation(d3[:, :, 0:1], ob3[:, :, 1:2], ACTF.Identity)
        nc.scalar.activation(d3[:, :, s - 1:s], ob3[:, :, s - 2:s - 1],
                             ACTF.Identity, scale=-1.0)
        nc.gpsimd.tensor_add(f3[:, :, 1:s - 1], ob3[:, :, 2:s],
                             ob3[:, :, 0:s - 2])
        nc.scalar.activation(f3[:, :, 0:1], ob3[:, :, 1:2], ACTF.Identity)
        nc.scalar.activation(f3[:, :, s - 1:s], ob3[:, :, s - 2:s - 1],
                             ACTF.Identity)
        nc.vector.scalar_tensor_tensor(o3[:, :, :], ob3[:, :, :], 2.0,
                                       f3[:, :, :], ALU.mult, ALU.add)
        u1 = mk(big, [F, hw], "u1", bf16)
        u2 = mk(big, [F, hw], "u2", bf16)
        u13 = u1[:].rearrange("p (h w) -> p h w", h=s)
        u23 = u2[:].rearrange("p (h w) -> p h w", h=s)
        nc.gpsimd.tensor_add(u13[:, 1:s - 1, :], d3[:, 0:s - 2, :],
                             d3[:, 2:s, :])
        nc.scalar.activation(u13[:, 0:1, :], d3[:, 1:2, :], ACTF.Identity)
        nc.scalar.activation(u13[:, s - 1:s, :], d3[:, s - 2:s - 1, :],
                             ACTF.Identity)
        nc.vector.scalar_tensor_tensor(u13[:, :, :], d3[:, :, :], 2.0,
                                       u13[:, :, :], ALU.mult, ALU.add)
        nc.vector.tensor_sub(u23[:, 1:s - 1, :], o3[:, 2:s, :],
                             o3[:, 0:s - 2, :])
        nc.scalar.activation(u23[:, 0:1, :], o3[:, 1:2, :], ACTF.Identity)
        nc.scalar.activation(u23[:, s - 1:s, :], o3[:, s - 2:s - 1, :],
                             ACTF.Identity, scale=-1.0)

        # ---- wave coefficient: so = sum(out) over space
        if so_prev is None:
            so = mk(smc, [F, 1], "so")
            nc.vector.tensor_reduce(so[:], out3, axis=AX.XY, op=ALU.add)
        else:
            so = so_prev
        so_bf = mk(smc, [F, 1], "so_bf", bf16)
        nc.vector.tensor_copy(so_bf[:], so[:])
        ps_c = mk(psp, [CPE, 1], "mini", bufs=1)
        mm(ps_c[:], w["c2w_w"][:], so_bf[:], True, True)
        coefv = mk(smc, [CPE, 1], "coefv")
        nc.scalar.activation(coefv[:], ps_c[:], ACTF.Identity,
                             bias=w["b8"][0:32, 3:4], scale=1.0 / S_C2W[l])
        coef96 = mk(smc, [96, 1], "coef96")
        nc.gpsimd.dma_start(coef96[64:96, :], coefv[:])
        nc.vector.tensor_scalar(w["weT"][64:96, :], w["w2ca"][64:96, :],
                                coef96[64:96, 0:1], None, ALU.mult)

        # ---- we/ge matmuls: drains on scalar, bn_stats on vector
        we_raw = mk(big, [128, hw], "weraw", bf16)
        ge_raw = mk(big, [128, hw], "geraw", bf16)
        we_st = mk(smc, [128, 6 * nch], "west")
        ge_st = mk(smc, [128, 6 * nch], "gest")
        for n in range(nch):
            c0 = n * cn0
            cn = cn0
            ps_we = mk(psp, [128, 512], "wg", bufs=2)
            mm(ps_we[:, :cn], w["weT"][:], Ucs[:, c0:c0 + cn], True, False)
            mm(ps_we[:, :cn], w["w2cbs"][:], sin_t[:, c0:c0 + cn], False, True)
            nc.scalar.activation(we_raw[:, c0:c0 + cn], ps_we[:, :cn],
                                 ACTF.Identity, scale=1.0 / S_W2C)
            nc.vector.bn_stats(we_st[:, n * 6:(n + 1) * 6],
                               we_raw[:, c0:c0 + cn])
        for n in range(nch):
            c0 = n * cn0
            cn = cn0
            ps_ge = mk(psp, [128, 512], "wg", bufs=2)
            mm(ps_ge[:, :cn], w["wA"][:], u1[:, c0:c0 + cn], True, False)
            mm(ps_ge[:, :cn], w["wB"][:], u2[:, c0:c0 + cn], False, True)
            nc.scalar.activation(ge_raw[:, c0:c0 + cn], ps_ge[:, :cn],
                                 ACTF.Identity, scale=1.0 / (S_G2C * 8.0))
            nc.vector.bn_stats(ge_st[:, n * 6:(n + 1) * 6],
                               ge_raw[:, c0:c0 + cn])

        # ---- instance-norm stats -> fold into skip/c1 weights
        def stats(st, nm):
            mv = mk(smc, [128, 2], f"mv{nm}")
            nc.vector.bn_aggr(mv[:], st[:])
            ve = mk(smc, [128, 1], f"ve{nm}")
            nc.vector.tensor_scalar(ve[:], mv[:, 1:2], 1e-5, None, ALU.add)
            nc.vector.reciprocal(ve[:], ve[:])
            rs = mk(smc, [128, 1], f"rs{nm}")
            nc.scalar.sqrt(rs[:], ve[:])
            nb = mk(smc, [128, 1], f"nb{nm}", bf16)
            nc.vector.scalar_tensor_tensor(nb[:], mv[:, 0:1], -1.0, rs[:],
                                           ALU.mult, ALU.mult)
            return rs, nb

        rs_ge, nb_ge = stats(ge_st, "g")
        rs_we, nb_we = stats(we_st, "w")

        skip_s = mk(gw, [128, 128], "skips", bf16)
        nc.vector.tensor_scalar(skip_s[:, 0:64], w["skip_k"][:, 0:64],
                                rs_ge[:], None, ALU.mult)
        nc.vector.tensor_scalar(skip_s[:, 64:128], w["skip_k"][:, 64:128],
                                rs_we[:], None, ALU.mult)
        c1_s = mk(gw, [128, 256], "c1s", bf16)
        nc.vector.tensor_scalar(c1_s[:, 0:128], w["c1_k"][:, 0:128],
                                rs_ge[:], None, ALU.mult)
        nc.vector.tensor_scalar(c1_s[:, 128:256], w["c1_k"][:, 128:256],
                                rs_we[:], None, ALU.mult)
        psF = mk(psp, [F, 1], "mini", bufs=1)
        mm(psF[:], w["skip_k"][:, 0:64], nb_ge[:], True, False)
        mm(psF[:], w["skip_k"][:, 64:128], nb_we[:], False, True)
        bsc = mk(smc, [F, 1], "bsc")
        nc.scalar.activation(bsc[:], psF[:], ACTF.Identity, bias=w["skipc3"][:],
                             scale=1.0 / S_SC)
        nc.vector.tensor_mul(bsc[:], bsc[:], leak64[:])
        psF2 = mk(psp, [128, 1], "mini", bufs=1)
        mm(psF2[:], w["c1_k"][:, 0:128], nb_ge[:], True, False)
        mm(psF2[:], w["c1_k"][:, 128:256], nb_we[:], False, True)
        c1b = mk(smc, [128, 1], "c1b")
        nc.scalar.activation(c1b[:], psF2[:], ACTF.Identity,
                             bias=w["b8"][:, 0:1], scale=1.0 / S_C1)

        # ---- residual: skip + c1/lrelu/c2/lrelu/c3, update out
        last_img = (l == L - 1 and c == 1)
        tnsum = mk(smc, [F, 8], "tnsum") if c == 0 else None
        for n in range(nch):
            c0 = n * cn0
            cn = cn0
            ps_s = mk(psp, [F, 512], "sc", bufs=2)
            mm(ps_s[:, :cn], skip_s[:, 0:64], ge_raw[:, c0:c0 + cn], True,
               False)
            mm(ps_s[:, :cn], skip_s[:, 64:128], we_raw[:, c0:c0 + cn], False,
               False)
            ps_1 = mk(psp, [128, 512], "pa", bufs=3)
            mm(ps_1[:, :cn], c1_s[:, 0:128], ge_raw[:, c0:c0 + cn], True,
               False)
            mm(ps_1[:, :cn], c1_s[:, 128:256], we_raw[:, c0:c0 + cn], False,
               True)
            a1 = mk(ab, [128, 512], "a1", bf16)
            nc.scalar.activation(a1[:, :cn], ps_1[:, :cn], ACTF.Identity,
                                 bias=c1b[:], scale=1.0 / S_C1)
            nc.vector.scalar_tensor_tensor(a1[:, :cn], a1[:, :cn], 0.2,
                                           a1[:, :cn], ALU.mult, ALU.max)
            ps_2 = mk(psp, [128, 512], "wg", bufs=2)
            mm(ps_2[:, :cn], w["c2_w"][:], a1[:, :cn], True, True)
            a2 = mk(ab, [128, 512], "a2", bf16)
            nc.scalar.activation(a2[:, :cn], ps_2[:, :cn], ACTF.Identity,
                                 bias=w["b8"][:, 1:2], scale=1.0 / S_C2)
            nc.vector.scalar_tensor_tensor(a2[:, :cn], a2[:, :cn], 0.2,
                                           a2[:, :cn], ALU.mult, ALU.max)
            mm(ps_s[:, :cn], w["c3_w"][:], a2[:, :cn], False, True)
            tn = mk(ab, [F, 512], "tn")
            if tnsum is not None:
                nc.scalar.activation(tn[:, :cn], ps_s[:, :cn], ACTF.Identity,
                                     bias=bsc[:], scale=leak_sc[:],
                                     accum_out=tnsum[:, n:n + 1])
            else:
                nc.scalar.activation(tn[:, :cn], ps_s[:, :cn], ACTF.Identity,
                                     bias=bsc[:], scale=leak_sc[:])
            nc.vector.tensor_add(out_t[:, c0:c0 + cn],
                                 out_t[:, c0:c0 + cn], tn[:, :cn])
            if ob_out is not None:
                nc.scalar.copy(ob_out[:, c0:c0 + cn], out_t[:, c0:c0 + cn])
            if last_img:
                ob = mk(ab, [F, 512], "ob", bf16)
                nc.scalar.copy(ob[:, :cn], out_t[:, c0:c0 + cn])
                ps_i = mk(psp, [3, 512], "mini", bufs=1)
                mm(ps_i[:, :cn], imgWT_bf[:], ob[:, :cn], True, True)
                imgc = mk(ab, [3, 512], "imgc")
                nc.scalar.activation(imgc[:, :cn], ps_i[:, :cn], ACTF.Identity,
                                     bias=imgb[:])
                nc.vector.tensor_scalar(imgc[:, :cn], imgc[:, :cn], -1.0, 1.0,
                                        ALU.max, ALU.min)
                nc.sync.dma_start(out_img[:, c0:c0 + cn], imgc[:, :cn])
        if tnsum is None:
            return None
        red = mk(smc, [F, 1], "sored")
        nc.vector.tensor_reduce(red[:], tnsum[:, 0:nch], axis=AX.X, op=ALU.add)
        so_next = mk(smc, [F, 1], "sonext")
        nc.vector.tensor_add(so_next[:], so[:], red[:])
        return so_next

    def transition(l):
        nonlocal out_t
        s = SIZES[l]
        s2 = 2 * s
        x = out_t[:].rearrange("p (h w) -> p h w", h=s)
        upv = mk(big, [F, s2 * s], "u1")
        v = upv[:].rearrange("p (h w) -> p h w", h=s2)
        # vertical polyphase (values scaled by 1/UC2)
        nc.vector.scalar_tensor_tensor(v[:, 2:s2 - 2:2, :], x[:, 0:s - 2, :],
                                       UA, x[:, 2:s, :], ALU.mult, ALU.add)
        nc.vector.scalar_tensor_tensor(v[:, 2:s2 - 2:2, :], x[:, 1:s - 1, :],
                                       UBC, v[:, 2:s2 - 2:2, :], ALU.mult,
                                       ALU.add)
        nc.vector.scalar_tensor_tensor(v[:, 3:s2 - 2:2, :], x[:, 2:s, :], UA,
                                       x[:, 0:s - 2, :], ALU.mult, ALU.add)
        nc.vector.scalar_tensor_tensor(v[:, 3:s2 - 2:2, :], x[:, 1:s - 1, :],
                                       UBC, v[:, 3:s2 - 2:2, :], ALU.mult,
                                       ALU.add)
        nc.vector.scalar_tensor_tensor(v[:, 0:1, :], x[:, 0:1, :], UE00,
                                       x[:, 1:2, :], ALU.mult, ALU.add)
        nc.vector.tensor_scalar(v[:, 1:2, :], x[:, 1:2, :], UA, None,
                                ALU.mult)
        nc.vector.scalar_tensor_tensor(v[:, 1:2, :], x[:, 0:1, :], UE1C,
                                       v[:, 1:2, :], ALU.mult, ALU.add)
        nc.vector.tensor_scalar(v[:, s2 - 2:s2 - 1, :], x[:, s - 1:s, :],
                                UE1C, None, ALU.mult)
        nc.vector.scalar_tensor_tensor(v[:, s2 - 2:s2 - 1, :],
                                       x[:, s - 2:s - 1, :], UA,
                                       v[:, s2 - 2:s2 - 1, :], ALU.mult,
                                       ALU.add)
        nc.vector.scalar_tensor_tensor(v[:, s2 - 1:s2, :], x[:, s - 1:s, :],
                                       UE2C, x[:, s - 2:s - 1, :], ALU.mult,
                                       ALU.add)
        # horizontal polyphase
        uph = mk(big, [F, s2 * s2], "fscratch")
        h3 = uph[:].rearrange("p (h w) -> p h w", h=s2)
        nc.vector.scalar_tensor_tensor(h3[:, :, 2:s2 - 2:2], v[:, :, 0:s - 2],
                                       UA, v[:, :, 2:s], ALU.mult, ALU.add)
        nc.vector.scalar_tensor_tensor(h3[:, :, 2:s2 - 2:2], v[:, :, 1:s - 1],
                                       UBC, h3[:, :, 2:s2 - 2:2], ALU.mult,
                                       ALU.add)
        nc.vector.scalar_tensor_tensor(h3[:, :, 3:s2 - 2:2], v[:, :, 2:s], UA,
                                       v[:, :, 0:s - 2], ALU.mult, ALU.add)
        nc.vector.scalar_tensor_tensor(h3[:, :, 3:s2 - 2:2], v[:, :, 1:s - 1],
                                       UBC, h3[:, :, 3:s2 - 2:2], ALU.mult,
                                       ALU.add)
        nc.vector.scalar_tensor_tensor(h3[:, :, 0:1], v[:, :, 0:1], UE00,
                                       v[:, :, 1:2], ALU.mult, ALU.add)
        nc.vector.tensor_scalar(h3[:, :, 1:2], v[:, :, 1:2], UA, None,
                                ALU.mult)
        nc.vector.scalar_tensor_tensor(h3[:, :, 1:2], v[:, :, 0:1], UE1C,
                                       h3[:, :, 1:2], ALU.mult, ALU.add)
        nc.vector.tensor_scalar(h3[:, :, s2 - 2:s2 - 1], v[:, :, s - 1:s],
                                UE1C, None, ALU.mult)
        nc.vector.scalar_tensor_tensor(h3[:, :, s2 - 2:s2 - 1],
                                       v[:, :, s - 2:s - 1], UA,
                                       h3[:, :, s2 - 2:s2 - 1], ALU.mult,
                                       ALU.add)
        nc.vector.scalar_tensor_tensor(h3[:, :, s2 - 1:s2], v[:, :, s - 1:s],
                                       UE2C, v[:, :, s - 2:s - 1], ALU.mult,
                                       ALU.add)
        newout = mk(big, [F, s2 * s2], "out_b" if l == 0 else "out_a")
        tacc = mk(smc, [F, 1], "tacc")
        nc.scalar.activation(newout[:], uph[:], ACTF.Copy,
                             scale=UC2 * UC2, accum_out=tacc[:])
        trans_acc[0] = tacc
        out_t = newout

    # ---------------- main schedule
    # Pre-emit phase A through layer-2-early so the PE fills the initial
    # barrier shadow; the rest (l2 late) pumps inside phase B of layer 0.
    pump(7 + 1 + 5 + 1)          # E0 + a2aE0 + L0 + a2aL0
    pump(7 + 1 + 5 + 1)          # E1 + a2aE1 + L1 + a2aL1
    pump(7 + 1)                  # E2 + a2aE2
    for l in range(L):
        hw = HWS[l]
        w = load_early(l)
        load_late(l, w)
        Ucs = mk(big, [96, hw], "ucs", bf16)
        nc.sync.dma_start(Ucs[64:96, :], ins[f"cw{l}"][:, :])
        sin_t = mk(big, [F, hw], "sin", bf16)
        surf(l, w, Ucs, sin_t)
        obf0 = mk(big, [F, hw], "obfa", bf16)
        nc.scalar.copy(obf0[:], out_t[:])
        obf1 = mk(big, [F, hw], "obfb", bf16)
        pump(2)
        so0 = call(l, 0, w, Ucs, sin_t,
                   trans_acc[0] if l > 0 else None, obf0, obf1)
        pump(2)
        call(l, 1, w, Ucs, sin_t, so0, obf1, None)
        pump(2)
        if l < L - 1:
            transition(l)

    for p in reversed(ctxs):
        p.__exit__(None, None, None)


_MODULE_CACHE = {}


def build_module():
    if "nc" in _MODULE_CACHE:
        return _MODULE_CACHE["nc"]
    from concourse import bacc, mybir
    import concourse.tile as tile

    nc = bacc.Bacc("TRN2", target_bir_lowering=False, debug=False,
                   num_devices=NCORE)
    f32 = mybir.dt.float32
    dts = {"f32": mybir.dt.float32, "bf16": mybir.dt.bfloat16,
           "fp8": mybir.dt.float8e4}
    ins = {}
    for name, shape, dt in INPUT_SHAPES:
        ins[name] = nc.dram_tensor(name, list(shape), dts[dt],
                                   kind="ExternalInput").ap()
    out_img = nc.dram_tensor("img", [3, HWS[-1]], f32,
                             kind="ExternalOutput").ap()
    with tile.TileContext(nc) as tc:
        build_kernel(tc, ins, out_img)
    nc.compile()
    _MODULE_CACHE["nc"] = nc
    return nc


# ---------------------------------------------------------------- host prep
def host_prep(inputs):
    """Full (unsharded) numpy inputs -> per-core in_maps."""
    lat = np.asarray(inputs["lat"], np.float32)
    leak = float(np.clip(np.asarray(inputs["leak_factor"]), 0.001, 1000.0))
    seed = np.asarray(inputs["seed"], np.float32)[0].reshape(F, 256)
    frac_W = np.asarray(inputs["frac_W"], np.float32)
    frac_b = np.asarray(inputs["frac_b"], np.float32)

    def T(W, fin, fout):
        W = np.asarray(W, np.float32)
        return W.reshape(L, LAT, fout, fin).transpose(0, 1, 3, 2)

    bt_T = T(inputs["bt_W"], 1024, F) * np.float32(S_BT)
    c2w_T = T(inputs["c2w_W"], F, CPE).copy()
    for l in range(L):
        c2w_T[l] *= np.float32(S_C2W[l] / HWS[l])
    w2c_T = T(inputs["w2c_W"], CPE + 2 * F, 2 * F) * np.float32(S_W2C)
    w2cA_T = np.ascontiguousarray(w2c_T[:, :, :CPE, :])
    w2cB_T = np.ascontiguousarray(w2c_T[:, :, CPE:, :]) * np.float32(E1)
    g2c_T = T(inputs["g2c_W"], 2 * F, 2 * F) * np.float32(S_G2C)
    skip_T = T(inputs["cc_skip_W"], 4 * F, F) * np.float32(S_SC)
    c1_T = T(inputs["cc_c1_W"], 4 * F, 2 * F) * np.float32(S_C1)
    c2_T = T(inputs["cc_c2_W"], 2 * F, 2 * F) * np.float32(S_C2)
    c3_T = T(inputs["cc_c3_W"], 2 * F, F) * np.float32(S_SC)

    # bias columns: (L, LAT, 8, 128); device reads b8[p, r] = col r value p
    bias_cols = np.zeros((L, LAT, 8, 128), np.float32)
    bias_cols[:, :, 0, :] = np.asarray(inputs["cc_c1_b"], np.float32)
    bias_cols[:, :, 1, :] = np.asarray(inputs["cc_c2_b"], np.float32)
    bias_cols[:, :, 2, 0:64] = np.asarray(inputs["bt_b"], np.float32)
    bias_cols[:, :, 3, 0:32] = np.asarray(inputs["c2w_b"], np.float32)
    bias_cols[:, :, 4, 0:64] = np.asarray(inputs["cc_skip_b"], np.float32)
    bias_cols[:, :, 5, 0:64] = np.asarray(inputs["cc_c3_b"], np.float32)
    bias_cols *= np.float32(S_B)

    def fout_shard(Tm, j, fo):
        blk = Tm[:, :, :, j * fo:(j + 1) * fo]  # (L, LAT, fin, fo)
        return np.ascontiguousarray(blk).reshape(L, LAT, -1)

    import ml_dtypes

    bf16 = ml_dtypes.bfloat16
    common = {
        "latT": np.ascontiguousarray(lat.T),
        "fracW": frac_W,
        "fracb": frac_b,
        "leak": np.full((1, 1), leak, np.float32),
        "seed": np.ascontiguousarray(seed),
        "imgWT": np.ascontiguousarray(np.asarray(inputs["img_W"],
                                                 np.float32).T),
        "imgb": np.asarray(inputs["img_b"], np.float32).reshape(3, 1),
    }
    for l in range(L):
        common[f"wb{l}"] = np.ascontiguousarray(
            np.asarray(inputs[f"wave_bias{l}"], np.float32)[0].reshape(
                F * 16, HWS[l])).astype(bf16)
        common[f"cw{l}"] = pos_enc_np(SIZES[l]).astype(bf16)

    fp8 = ml_dtypes.float8_e4m3

    def interleave8(arr):
        # (L, 256, S) f32 -> (L, 128, 2S) fp8: per 512-block (matching the
        # device chunk lattice), row p holds [k-tile0 block | k-tile1 block]
        Lx, K, S = arr.shape
        out = np.empty((Lx, 128, 2 * S), np.float32)
        for c0 in range(0, S, 2048):
            for n0 in range(c0, min(c0 + 2048, S), 512):
                nn = min(512, S - n0)
                out[:, :, 2 * n0:2 * n0 + nn] = arr[:, 0:128, n0:n0 + nn]
                out[:, :, 2 * n0 + nn:2 * n0 + 2 * nn] = \
                    arr[:, 128:256, n0:n0 + nn]
        return out.astype(fp8)

    in_maps = []
    for j in range(NCORE):
        wwsB = np.ascontiguousarray(
            bt_T[:, :, j * 128:(j + 1) * 128, :]).reshape(L, LAT, N_BT)
        wwsE = np.concatenate([
            fout_shard(c2w_T, j, 4),
            fout_shard(w2cA_T, j, 16),
            fout_shard(w2cB_T, j, 16),
            fout_shard(g2c_T, j, 16),
            np.ascontiguousarray(bias_cols[:, :, j, :]),
        ], axis=2)
        assert wwsE.shape == (L, LAT, N_RESTE), wwsE.shape
        wwsL = np.concatenate([
            fout_shard(skip_T, j, 8),
            fout_shard(c1_T, j, 16),
            fout_shard(c2_T, j, 16),
            fout_shard(c3_T, j, 8),
        ], axis=2)
        assert wwsL.shape == (L, LAT, SCOLS_L), wwsL.shape
        m = dict(common)
        m["wwsB"] = interleave8(wwsB)
        m["wwsE"] = wwsE.astype(bf16)
        m["wwsL"] = wwsL.astype(bf16)
        in_maps.append(m)
    return in_maps


def kernel(**inputs):
    from concourse.bass_utils import run_bass_kernel_spmd

    nc = build_module()
    in_maps = host_prep(inputs)
    res = run_bass_kernel_spmd(nc, in_maps, list(range(NCORE)))
    imgs = [res.results[b]["img"].reshape(3, SIZES[-1], SIZES[-1])
            for b in range(B)]
    return np.stack(imgs).astype(np.float32)


if __name__ == "__main__":
    nc = build_module()
    print("module built ok; instructions:",
          sum(len(bb.instructions) for bb in nc.main_func.blocks))
